# revision 51
# baseline (speedup 1.0000x reference)
"""Distributed Trainium2 Bass kernel for nn_BlockMoE (B=2,T=2048,D=1024,H=16,E=8,K=2).

Sharding (SPMD, one shared instruction stream; all per-core variation via input shards):
  - LN1/LN2/router/output: token-sharded (core c owns global tokens [512c, 512c+512))
  - attention: head-sharded (core c owns heads {2c, 2c+1} via wq/wk/wv column shards)
  - MoE: expert-sharded (core c owns expert c), dense-equivalent compute with gate masking
Collectives: AG(xln1T f32r) -> RS(xoT partials f32r) -> AG(xln2 bf16) + AG(probs f32)
             -> RS(MoE partials bf16).
Attention chain in float32r (TF32-like, full TensorE rate) to keep router top-2
selection faithful; expert MLP in bf16.
"""
import os
import sys
import types

import numpy as np

sys.path.insert(0, '/opt/trn_rl_repo')
sys.path.insert(0, '/opt/trn_rl_repo/concourse')

import concourse.bacc as bacc
import concourse.bass as bass
import concourse.mybir as mybir
import concourse.tile as tile
from concourse import bass_utils

# ---------------------------------------------------------------- trace shim
# bass_utils under BASS_TRACE imports antenv.axon_hooks, absent in this image.
try:
    import antenv
    if not hasattr(antenv, 'axon_hooks'):
        m = types.ModuleType('antenv.axon_hooks')
        m._hook = None
        m.set_axon_ntff_profile_hook = lambda h: setattr(m, '_hook', h)
        m.get_axon_ntff_profile_hook = lambda: m._hook
        sys.modules['antenv.axon_hooks'] = m
        antenv.axon_hooks = m
    if os.environ.get('BASS_TRACE'):
        from antenv.axon_hooks import get_axon_ntff_profile_hook, set_axon_ntff_profile_hook
        if get_axon_ntff_profile_hook() is None:
            from trn_agent_boot.trn_boot import _ntff_profile_via_ctypes
            set_axon_ntff_profile_hook(_ntff_profile_via_ctypes('/opt/axon/libaxon_pjrt.so'))
except Exception:
    pass

B, T, D, H, E, TOPK = 2, 2048, 1024, 16, 8, 2
F = 4 * D
HD = D // H          # 64
NC = 8               # cores
TOK = B * T          # 4096
OWN = TOK // NC      # 512 tokens per core
HPC = H // NC        # 2 heads per core
EPS = 1e-5

f32 = mybir.dt.float32
f32r = mybir.dt.float32r
bf16 = mybir.dt.bfloat16

RG = [list(range(NC))]


def build_nc(debug=False):
    nc = bacc.Bacc("TRN2", num_devices=NC)

    # ---------------- parameters (per-core shards prepared by host) ----------
    xT_p = nc.dram_tensor("xT", [D, OWN], f32r, kind="ExternalInput")          # own tokens, transposed
    wq_p = nc.dram_tensor("wq", [D, HPC * HD], f32r, kind="ExternalInput")     # own heads' q cols
    wk_p = nc.dram_tensor("wk", [D, HPC * HD], f32r, kind="ExternalInput")
    wv_p = nc.dram_tensor("wv", [D, HPC * HD], f32r, kind="ExternalInput")
    wproj_p = nc.dram_tensor("wproj", [D, D], f32r, kind="ExternalInput")  # full (replicated)
    router_p = nc.dram_tensor("router_w", [D, E], f32r, kind="ExternalInput")
    ln1_p = nc.dram_tensor("ln1_w", [128, D // 128], f32, kind="ExternalInput")   # [p, i] = w[i*128+p]
    ln2_p = nc.dram_tensor("ln2_w", [128, D // 128], f32, kind="ExternalInput")
    w1_p = nc.dram_tensor("w1", [D, F], f32, kind="ExternalInput")             # own expert
    w2_p = nc.dram_tensor("w2", [F, D], f32, kind="ExternalInput")
    ident_p = nc.dram_tensor("ident", [128, 128], f32r, kind="ExternalInput")
    ones_p = nc.dram_tensor("ones", [128, 128], f32r, kind="ExternalInput")
    causal_p = nc.dram_tensor("causal", [2 * 128, 256], f32, kind="ExternalInput")  # [sub*128+p, kk]
    esel_p = nc.dram_tensor("esel", [128, E], f32, kind="ExternalInput")       # one-hot row c, replicated
    tokp1_p = nc.dram_tensor("tokp1", [16, 256], mybir.dt.int16, kind="ExternalInput")  # token id + 1

    out_p = nc.dram_tensor("out", [OWN, D], f32, kind="ExternalOutput")
    dbg = {}
    if debug:
        for name, shape, dt_ in [
            ("d_xln1T", [D, OWN], f32), ("d_q", [128, 8 * 512], f32), ("d_k", [128, 8 * 512], f32),
            ("d_v", [128, 32 * 132], f32), ("d_oT", [128, 8 * 512], f32), ("d_xoT", [D, OWN], f32),
            ("d_xln2T", [D, OWN], f32), ("d_probs", [OWN, E], f32), ("d_rsum", [128, 64], f32),
            ("d_attnT", [128, 16 * 512], f32), ("d_selg", [TOK, 2], f32),
            ("d_ids", [32, 128], mybir.dt.int16), ("d_gs", [1, 1536], mybir.dt.float16),
            ("d_ns", [OWN, D], bf16),
        ]:
            dbg[name] = nc.dram_tensor(name, shape, dt_, kind="ExternalOutput")

    KT = D // 128  # 8 contraction tiles over D

    with tile.TileContext(nc) as tc:
        # ---------------- DRAM bounce buffers ------------------------------
        with tc.tile_pool(name="dram", bufs=1, space="DRAM") as dram:
            ag1_in = dram.tile([D, OWN], f32r)                    # xln1T contribution
            ag1_out = dram.tile([NC * D, OWN], f32r, addr_space="Shared")
            a2ao_in = dram.tile([NC * 128, OWN], f32r)            # my heads' oT per owner block
            a2ao_out = dram.tile([NC * 128, OWN], f32r)           # full oT for my tokens
            agx_in = dram.tile([OWN, D], bf16)                    # xln2 rows bf16
            agx_out = dram.tile([TOK, D], bf16, addr_space="Shared")
            agp_in = dram.tile([TOK, 2], f32)                     # own toks x all experts [sel, gate]
            agp_out = dram.tile([TOK, 2], f32)
            idx_dram = dram.tile([32, 128], mybir.dt.int16)       # ids bounce (g in 0:16, s in 16:32)
            gs_dram = dram.tile([1, 1536], mybir.dt.float16)      # gate-per-slot bounce
            partial = dram.tile([TOK + 128, D], bf16)             # scatter table (+trash rows)
            rs2_out = dram.tile([OWN, D], bf16)
            x2f_dram = dram.tile([OWN, D], f32)                   # LN2 rows f32 (for P6)
            rt_dram = dram.tile([16, 128], f32r)                  # recip flatten bounce
            gt_dram = dram.tile([4, 128], f32r)                   # gate flatten bounce

            # ---------------- persistent SBUF ------------------------------
            with tc.tile_pool(name="persist", bufs=1) as pp:
                ident = pp.tile([128, 128], f32r)
                nc.sync.dma_start(ident[:], ident_p[:])
                ident_bf = pp.tile([128, 128], bf16)
                nc.vector.tensor_copy(ident_bf[:], ident[:])
                ones = pp.tile([128, 128], f32r)
                nc.sync.dma_start(ones[:], ones_p[:])
                causal = pp.tile([128, 2, 256], f32)
                nc.sync.dma_start(causal[:], causal_p[:].rearrange("(s p) k -> p s k", p=128))
                ln2w = pp.tile([128, KT], f32)
                nc.sync.dma_start(ln2w[:], ln2_p[:])
                esel = pp.tile([128, E], f32)
                nc.sync.dma_start(esel[:], esel_p[:])
                xln2 = pp.tile([128, KT, OWN], f32r)              # LN2 output (own)

                pa_cm = tc.tile_pool(name="phaseA", bufs=1)
                pa = pa_cm.__enter__()

                # ---------- helper: layernorm in [feat, tok] layout ----------
                def layer_norm_T(src, dst, wcol, psum_pool, sbuf_pool):
                    """src, dst: [128, KT, OWN] (f32-readable); wcol [128, KT]."""
                    sum_ps = psum_pool.tile([1, OWN], f32, space="PSUM")
                    sq_ps = psum_pool.tile([1, OWN], f32, space="PSUM")
                    for kt in range(KT):
                        nc.tensor.matmul(sum_ps[:], ones[:, :1], src[:, kt, :],
                                         start=(kt == 0), stop=(kt == KT - 1))
                    for kt in range(KT):
                        sqt = sbuf_pool.tile([128, OWN], f32r, tag="lnsq", bufs=1)
                        nc.vector.tensor_tensor(out=sqt[:], in0=src[:, kt, :], in1=src[:, kt, :],
                                                op=mybir.AluOpType.mult)
                        nc.tensor.matmul(sq_ps[:], ones[:, :1], sqt[:],
                                         start=(kt == 0), stop=(kt == KT - 1))
                    mu = sbuf_pool.tile([1, OWN], f32, tag="lnmu")
                    nc.vector.tensor_scalar_mul(mu[:], sum_ps[:], 1.0 / D)
                    msq = sbuf_pool.tile([1, OWN], f32, tag="lnmsq")
                    nc.vector.tensor_scalar_mul(msq[:], sq_ps[:], 1.0 / D)
                    mu2 = sbuf_pool.tile([1, OWN], f32, tag="lnmu2")
                    nc.vector.tensor_tensor(out=mu2[:], in0=mu[:], in1=mu[:], op=mybir.AluOpType.mult)
                    var = sbuf_pool.tile([1, OWN], f32, tag="lnvar")
                    nc.vector.tensor_sub(var[:], msq[:], mu2[:])
                    nc.vector.tensor_scalar_add(var[:], var[:], EPS)
                    std = sbuf_pool.tile([1, OWN], f32, tag="lnstd")
                    nc.scalar.activation(std[:], var[:], mybir.ActivationFunctionType.Sqrt)
                    rstd = sbuf_pool.tile([1, OWN], f32, tag="lnrstd")
                    nc.vector.reciprocal(rstd[:], std[:])
                    mur = sbuf_pool.tile([1, OWN], f32r, tag="lnmur")
                    nc.vector.tensor_copy(mur[:], mu[:])
                    rstdr = sbuf_pool.tile([1, OWN], f32r, tag="lnrstdr")
                    nc.vector.tensor_copy(rstdr[:], rstd[:])
                    mu_b = psum_pool.tile([128, OWN], f32, space="PSUM")
                    rstd_b = psum_pool.tile([128, OWN], f32, space="PSUM")
                    nc.tensor.matmul(mu_b[:], ones[:1, :], mur[:], start=True, stop=True)
                    nc.tensor.matmul(rstd_b[:], ones[:1, :], rstdr[:], start=True, stop=True)
                    for kt in range(KT):
                        t1 = sbuf_pool.tile([128, OWN], f32, tag="lnt1")
                        nc.vector.tensor_sub(t1[:], src[:, kt, :], mu_b[:])
                        t2 = sbuf_pool.tile([128, OWN], f32, tag="lnt2")
                        nc.vector.tensor_tensor(out=t2[:], in0=t1[:], in1=rstd_b[:], op=mybir.AluOpType.mult)
                        nc.vector.tensor_scalar_mul(dst[:, kt, :], t2[:], wcol[:, kt:kt + 1])

                # ================= P0: LN1 + AG ===========================
                with tc.tile_pool(name="p0sb", bufs=1) as p0sb, \
                     tc.tile_pool(name="p0ps", bufs=1, space="PSUM") as p0ps:
                    ln1w = p0sb.tile([128, KT], f32)
                    nc.sync.dma_start(ln1w[:], ln1_p[:])
                    xt = p0sb.tile([128, KT, OWN], f32r)          # own xT
                    nc.sync.dma_start(xt[:], xT_p[:].rearrange("(kt p) t -> p kt t", p=128))
                    xln1 = p0sb.tile([128, KT, OWN], f32r)
                    layer_norm_T(xt, xln1, ln1w, p0ps, p0sb)
                    nc.sync.dma_start(ag1_in[:].rearrange("(kt p) t -> p kt t", p=128), xln1[:])
                    if debug:
                        nc.sync.dma_start(dbg["d_xln1T"][:].rearrange("(kt p) t -> p kt t", p=128), xln1[:].bitcast(f32))
                nc.gpsimd.collective_compute(
                    "AllGather", mybir.AluOpType.bypass, replica_groups=RG,
                    ins=[ag1_in[:].opt()], outs=[ag1_out[:].opt()])
                # zero the MoE scatter table (runs early, overlaps attention)
                with tc.tile_pool(name="zpool", bufs=1) as zp:
                    zt = zp.tile([128, D], bf16)
                    nc.vector.memset(zt[:], 0)
                    for zi in range((TOK + 128) // 128):
                        nc.sync.dma_start(partial[zi * 128:(zi + 1) * 128, :], zt[:])

                # ================= P1: qkv (own 2 heads, all tokens) =======

                with tc.tile_pool(name="attn_sb", bufs=1) as asb:
                    p1ps_cm = tc.tile_pool(name="p1ps", bufs=1, space="PSUM")
                    aps = p1ps_cm.__enter__()
                    tps = aps
                    wqp_cm = tc.tile_pool(name="wqp", bufs=1)
                    wqp = wqp_cm.__enter__()
                    wq = wqp.tile([128, KT, HPC * HD], f32r)
                    nc.sync.dma_start(wq[:], wq_p[:].rearrange("(kt p) m -> p kt m", p=128))
                    wk = wqp.tile([128, KT, HPC * HD], f32r)
                    nc.sync.dma_start(wk[:], wk_p[:].rearrange("(kt p) m -> p kt m", p=128))
                    wv = wqp.tile([128, KT, HPC * HD], f32r)
                    nc.sync.dma_start(wv[:], wv_p[:].rearrange("(kt p) m -> p kt m", p=128))
                    q_sb = asb.tile([128, NC, 512], f32r)   # [2h*64, rblk, tok]
                    k_sb = asb.tile([128, NC, 512], f32r)
                    v_sb = asb.tile([128, 32, 132], f32r)   # [tok128, t-tile, h*65+{64 feat, 1 ones}]
                    for _t in range(32):
                        nc.vector.tensor_copy(v_sb[:, _t, 64:65], ones[:, :1])
                        nc.vector.tensor_copy(v_sb[:, _t, 129:130], ones[:, :1])
                    for r in range(NC):
                        xg1_r = wqp.tile([128, KT, 512], f32r, tag="xg1", bufs=2)
                        nc.sync.dma_start(
                            xg1_r[:], ag1_out[r * D:(r + 1) * D, :].rearrange("(kt p) t -> p kt t", p=128))
                        q_ps = aps.tile([128, 512], f32, space="PSUM", tag="qkv", bufs=3)
                        for kt in range(KT):
                            nc.tensor.matmul(q_ps[:HPC * HD, :], wq[:, kt, :], xg1_r[:, kt, :],
                                             start=(kt == 0), stop=(kt == KT - 1))
                        nc.vector.tensor_copy(q_sb[:HPC * HD, r, :], q_ps[:HPC * HD, :])
                        k_ps = aps.tile([128, 512], f32, space="PSUM", tag="qkv", bufs=3)
                        for kt in range(KT):
                            nc.tensor.matmul(k_ps[:HPC * HD, :], wk[:, kt, :], xg1_r[:, kt, :],
                                             start=(kt == 0), stop=(kt == KT - 1))
                        nc.vector.tensor_copy(k_sb[:HPC * HD, r, :], k_ps[:HPC * HD, :])
                        v_ps = aps.tile([128, 512], f32, space="PSUM", tag="qkv", bufs=3)
                        for kt in range(KT):
                            nc.tensor.matmul(v_ps[:HPC * HD, :], wv[:, kt, :], xg1_r[:, kt, :],
                                             start=(kt == 0), stop=(kt == KT - 1))
                        vT_sb = asb.tile([128, 512], f32r, tag="vT", bufs=2)
                        nc.vector.tensor_copy(vT_sb[:HPC * HD, :], v_ps[:HPC * HD, :])
                        # transpose v to [tok, feat]; interleave ones col per head
                        for tt in range(4):
                            v_tps = tps.tile([128, 128], f32r, space="PSUM", tag="vtr", bufs=3)
                            nc.tensor.transpose(v_tps[:], vT_sb[:, tt * 128:(tt + 1) * 128], ident[:])
                            nc.vector.tensor_copy(v_sb[:, r * 4 + tt, 0:64], v_tps[:, 0:64])
                            nc.vector.tensor_copy(v_sb[:, r * 4 + tt, 65:129], v_tps[:, 64:128])
                    if debug:
                        nc.sync.dma_start(dbg["d_q"][:].rearrange("p (r t) -> p r t", r=NC), q_sb[:].bitcast(f32))
                        nc.sync.dma_start(dbg["d_k"][:].rearrange("p (r t) -> p r t", r=NC), k_sb[:].bitcast(f32))
                        nc.sync.dma_start(dbg["d_v"][:].rearrange("p (r t) -> p r t", r=32, t=132), v_sb[:].bitcast(f32))

                    wqp_cm.__exit__(None, None, None)
                    p1ps_cm.__exit__(None, None, None)
                    p2ps_cm = tc.tile_pool(name="p2ps", bufs=1, space="PSUM")
                    aps = p2ps_cm.__enter__()
                    tps = aps
                    # ============= P2: scores/softmax/AV per (b, h) =========
                    oT_sb = asb.tile([128, NC, 512], f32r)   # [2h*64, rblk, tok]
                    for b in range(B):
                        for h in range(HPC):
                            hs = h * HD
                            PT = asb.tile([128, 16, 512], f32r, tag="attnT", bufs=1)
                            for qc in range(4):
                                rq = b * 4 + qc
                                nkt = 4 * qc + 4
                                for kt in range(nkt):
                                    u = kt // 2
                                    ru = b * 4 + u // 2
                                    ik = (u % 2) * 256 + (kt % 2) * 128
                                    qs = max(0, u * 256 - qc * 512)
                                    s_ps = aps.tile([128, 512], f32, space="PSUM", tag="score", bufs=3)
                                    nc.tensor.matmul(s_ps[:, qs:512],
                                                     k_sb[hs:hs + HD, ru, ik:ik + 128],
                                                     q_sb[hs:hs + HD, rq, qs:512],
                                                     start=True, stop=True)
                                    dq = u * 256 - qc * 512   # diag block q-col start
                                    if 0 <= dq < 512:
                                        nc.vector.tensor_add(s_ps[:, dq:dq + 256], s_ps[:, dq:dq + 256],
                                                             causal[:, kt % 2, :])
                                    nc.scalar.activation(PT[:, kt, qs:512], s_ps[:, qs:512],
                                                         mybir.ActivationFunctionType.Exp, scale=0.125)
                                o_ps = aps.tile([128, 512], f32, space="PSUM", tag="avps", bufs=3)
                                for kt in range(nkt):
                                    qs = max(0, (kt // 2) * 256 - qc * 512)
                                    nc.tensor.matmul(
                                        o_ps[:HD + 1, qs:512],
                                        v_sb[:, b * 16 + kt, h * 65:h * 65 + 65],
                                        PT[:, kt, qs:512],
                                        start=(kt == 0), stop=(kt == nkt - 1))
                                rs_row = asb.tile([1, 512], f32, tag="rsrow", bufs=2)
                                nc.vector.reciprocal(rs_row[:], o_ps[HD:HD + 1, :])
                                rcp_row = asb.tile([1, 512], f32r, tag="rcprow", bufs=2)
                                nc.vector.tensor_copy(rcp_row[:], rs_row[:])
                                rb_ps = aps.tile([128, 512], f32, space="PSUM", tag="rbcast", bufs=1)
                                nc.tensor.matmul(rb_ps[:], ones[:1, :], rcp_row[:], start=True, stop=True)
                                rb_sb = asb.tile([128, 512], f32, tag="rbsb", bufs=2)
                                nc.vector.tensor_copy(rb_sb[:], rb_ps[:])
                                nc.vector.tensor_tensor(
                                    out=oT_sb[hs:hs + HD, b * 4 + qc, :],
                                    in0=o_ps[:HD, :], in1=rb_sb[:HD, :], op=mybir.AluOpType.mult)
                    if debug:
                        nc.sync.dma_start(dbg["d_oT"][:].rearrange("p (r t) -> p r t", r=NC), oT_sb[:].bitcast(f32))

                    p2ps_cm.__exit__(None, None, None)
                    # ============= P3: ship oT blocks to token owners =======
                    for r in range(NC):
                        nc.sync.dma_start(a2ao_in[r * 128:(r + 1) * 128, :], oT_sb[:, r, :])
                pa_cm.__exit__(None, None, None)
                PERCAP = 96
                CAP = 16 * PERCAP                                  # 1536 slots
                w1p_cm = tc.tile_pool(name="w1p", bufs=1)
                w1p = w1p_cm.__enter__()
                w1b = w1p.tile([128, KT, F], bf16)       # [Dpart, kt, F]
                with tc.tile_pool(name="wconv1", bufs=2) as wcp:
                    for kt in range(KT):
                        for ch in range(2):
                            wt = wcp.tile([128, 2048], f32, tag="wc32")
                            nc.sync.dma_start(wt[:], w1_p[kt * 128:(kt + 1) * 128,
                                                          ch * 2048:(ch + 1) * 2048])
                            nc.vector.tensor_copy(w1b[:, kt, ch * 2048:(ch + 1) * 2048], wt[:])
                nc.gpsimd.collective_compute(
                    "AllToAll", mybir.AluOpType.bypass, replica_groups=RG,
                    ins=[a2ao_in[:].opt()], outs=[a2ao_out[:].opt()])

                # ================= P4: residual + LN2 + router ==============
                router_w = pp.tile([128, KT, E], f32r)
                nc.sync.dma_start(router_w[:], router_p[:].rearrange("(kt p) e -> p kt e", p=128))
                with tc.tile_pool(name="p4sb", bufs=1) as p4sb, \
                     tc.tile_pool(name="p4ps", bufs=1, space="PSUM") as p4ps:
                    xres = p4sb.tile([128, KT, OWN], f32r)
                    p4o_cm = tc.tile_pool(name="p4o", bufs=1)
                    p4o = p4o_cm.__enter__()
                    oT_full = p4o.tile([128, KT, OWN], f32r)
                    nc.sync.dma_start(oT_full[:], a2ao_out[:].rearrange("(kt p) t -> p kt t", p=128))
                    for dm in range(KT):
                        pj_ps = p4ps.tile([128, OWN], f32, space="PSUM", tag="proj", bufs=2)
                        for kt in range(KT):
                            wpj_t = p4o.tile([128, 128], f32r, tag="wpjt", bufs=2)
                            nc.sync.dma_start(wpj_t[:], wproj_p[kt * 128:(kt + 1) * 128,
                                                                dm * 128:(dm + 1) * 128])
                            nc.tensor.matmul(pj_ps[:], wpj_t[:], oT_full[:, kt, :],
                                             start=(kt == 0), stop=(kt == KT - 1))
                        xt_t = p4sb.tile([128, OWN], f32r, tag="xtt", bufs=2)
                        nc.sync.dma_start(xt_t[:], xT_p[dm * 128:(dm + 1) * 128, :])
                        nc.vector.tensor_add(xres[:, dm, :], xt_t[:], pj_ps[:])
                    p4o_cm.__exit__(None, None, None)
                    if debug:
                        nc.sync.dma_start(dbg["d_xoT"][:].rearrange("(kt p) t -> p kt t", p=128), xres[:].bitcast(f32))
                    layer_norm_T(xres, xln2, ln2w, p4ps, p4sb)
                    if debug:
                        nc.sync.dma_start(dbg["d_xln2T"][:].rearrange("(kt p) t -> p kt t", p=128), xln2[:].bitcast(f32))
                    # transpose xln2 -> token-row layout (bf16 for gather table, f32 for P6)
                    x2row = p4sb.tile([128, 4, D], bf16)
                    for kt in range(KT):
                        for tt in range(4):
                            x2_tps = p4ps.tile([128, 128], f32r, space="PSUM", tag="x2tr", bufs=1)
                            nc.tensor.transpose(x2_tps[:], xln2[:, kt, tt * 128:(tt + 1) * 128], ident[:])
                            nc.vector.tensor_copy(x2row[:, tt, kt * 128:(kt + 1) * 128], x2_tps[:])
                            x2f_t = p4sb.tile([128, 128], f32, tag="x2ft", bufs=2)
                            nc.vector.tensor_copy(x2f_t[:], x2_tps[:])
                            nc.sync.dma_start(x2f_dram[tt * 128:(tt + 1) * 128, kt * 128:(kt + 1) * 128],
                                              x2f_t[:])
                    nc.sync.dma_start(agx_in[:].rearrange("(tt p) d2 -> p tt d2", p=128), x2row[:])
                    # router: logits [tok, E] for own tokens
                    probs = p4sb.tile([128, 4, E], f32)
                    for mt in range(4):
                        lg_ps = p4ps.tile([128, E], f32, space="PSUM", tag="router", bufs=1)
                        for kt in range(KT):
                            nc.tensor.matmul(lg_ps[:], xln2[:, kt, mt * 128:(mt + 1) * 128],
                                             router_w[:, kt, :], start=(kt == 0), stop=(kt == KT - 1))
                        pex = p4sb.tile([128, E], f32, tag="pex", bufs=2)
                        nc.scalar.activation(pex[:], lg_ps[:], mybir.ActivationFunctionType.Exp)
                        psum_r = p4sb.tile([128, 1], f32, tag="psr", bufs=2)
                        nc.vector.tensor_reduce(psum_r[:], pex[:], axis=mybir.AxisListType.X,
                                                op=mybir.AluOpType.add)
                        prcp = p4sb.tile([128, 1], f32, tag="prcp", bufs=2)
                        nc.vector.reciprocal(prcp[:], psum_r[:])
                        nc.vector.tensor_scalar_mul(probs[:, mt, :], pex[:], prcp[:])
                    # own-token [sel, gate] for EVERY expert, A2A-dispatched
                    selg = p4sb.tile([128, E, 4, 2], f32)
                    for mt in range(4):
                        m8 = p4sb.tile([128, 8], f32, tag="m8", bufs=2)
                        nc.vector.max(out=m8[:], in_=probs[:, mt, :])
                        den = p4sb.tile([128, 1], f32, tag="den", bufs=2)
                        nc.vector.tensor_add(den[:], m8[:, 0:1], m8[:, 1:2])
                        rden = p4sb.tile([128, 1], f32, tag="rden", bufs=2)
                        nc.vector.reciprocal(rden[:], den[:])
                        for e in range(E):
                            pe = probs[:, mt, e:e + 1]
                            nc.vector.tensor_tensor(out=selg[:, e, mt, 0:1], in0=pe, in1=m8[:, 1:2],
                                                    op=mybir.AluOpType.is_ge)
                            g1 = p4sb.tile([128, 1], f32, tag="g1", bufs=2)
                            nc.vector.tensor_tensor(out=g1[:], in0=pe, in1=rden[:],
                                                    op=mybir.AluOpType.mult)
                            nc.vector.tensor_tensor(out=selg[:, e, mt, 1:2], in0=g1[:],
                                                    in1=selg[:, e, mt, 0:1],
                                                    op=mybir.AluOpType.mult)
                    nc.sync.dma_start(agp_in[:].rearrange("(e mt p) o -> p e mt o", p=128, mt=4), selg[:])
                    if debug:
                        nc.sync.dma_start(dbg["d_probs"][:].rearrange("(mt p) e -> p mt e", p=128), probs[:])
                nc.gpsimd.collective_compute(
                    "AllToAll", mybir.AluOpType.bypass, replica_groups=RG,
                    ins=[agp_in[:].opt()], outs=[agp_out[:].opt()])
                nc.gpsimd.collective_compute(
                    "AllGather", mybir.AluOpType.bypass, replica_groups=RG,
                    ins=[agx_in[:].opt()], outs=[agx_out[:].opt()])
                moe_w_cm = tc.tile_pool(name="moe_w", bufs=1)
                moe_w = moe_w_cm.__enter__()
                w2b = moe_w.tile([128, F // 128, D], bf16)  # [Fpart, ft, D]
                with tc.tile_pool(name="wconv2", bufs=2) as wcp2:
                    for ft in range(F // 128):
                        wt = wcp2.tile([128, 1024], f32, tag="wc32b")
                        nc.sync.dma_start(wt[:], w2_p[ft * 128:(ft + 1) * 128, :])
                        nc.vector.tensor_copy(w2b[:, ft, :], wt[:])
                # ================= P5: routed expert (own expert) ===========
                # ---- index build: compact token list for own expert ----
                with tc.tile_pool(name="idx_sb", bufs=1) as isb:
                    selw = isb.tile([16, 256], f32)
                    nc.sync.dma_start(selw[:], agp_out[:, 0:1].rearrange("(p j) o -> p (j o)", p=16))
                    gatew = isb.tile([16, 256], f32)
                    nc.sync.dma_start(gatew[:], agp_out[:, 1:2].rearrange("(p j) o -> p (j o)", p=16))
                    tokp1 = isb.tile([16, 256], mybir.dt.int16)
                    nc.sync.dma_start(tokp1[:], tokp1_p[:])
                    incl = isb.tile([16, 256], f32)
                    nc.vector.tensor_tensor_scan(incl[:], selw[:], selw[:], 0.0,
                                                 op0=mybir.AluOpType.add, op1=mybir.AluOpType.bypass)
                    pos = isb.tile([16, 256], f32)
                    nc.vector.tensor_sub(pos[:], incl[:], selw[:])
                    # pos_m = pos*sel + sel - 1  (-1 for unselected), clamped
                    nc.vector.tensor_tensor(out=pos[:], in0=pos[:], in1=selw[:], op=mybir.AluOpType.mult)
                    nc.vector.tensor_add(pos[:], pos[:], selw[:])
                    nc.vector.tensor_scalar_add(pos[:], pos[:], -1.0)
                    nc.vector.tensor_scalar_min(pos[:], pos[:], float(PERCAP - 1))
                    pos16 = isb.tile([16, 256], mybir.dt.int16)
                    nc.vector.tensor_copy(pos16[:], pos[:])
                    idbuf = isb.tile([16, PERCAP], mybir.dt.int16)
                    nc.gpsimd.local_scatter(idbuf[:], tokp1[:], pos16[:], channels=16,
                                            num_elems=PERCAP, num_idxs=256)
                    gate16 = isb.tile([16, 256], mybir.dt.float16)
                    nc.vector.tensor_copy(gate16[:], gatew[:])
                    gatebuf = isb.tile([16, PERCAP], mybir.dt.float16)
                    nc.gpsimd.local_scatter(gatebuf[:], gate16[:], pos16[:], channels=16,
                                            num_elems=PERCAP, num_idxs=256)
                    # fixups in f32: gather ids = max(id-1, 0); scatter ids = (id==0) ? TOK+p : id-1
                    idf = isb.tile([16, PERCAP], f32)
                    nc.vector.tensor_copy(idf[:], idbuf[:])
                    ise = isb.tile([16, PERCAP], f32)
                    nc.vector.tensor_scalar(ise[:], idf[:], 0.0, scalar2=None,
                                            op0=mybir.AluOpType.is_equal)
                    nc.vector.tensor_scalar_add(idf[:], idf[:], -1.0)
                    gth = isb.tile([16, PERCAP], f32)
                    nc.vector.tensor_scalar_max(gth[:], idf[:], 0.0)
                    idsg16 = isb.tile([16, PERCAP], mybir.dt.int16)
                    nc.vector.tensor_copy(idsg16[:], gth[:])
                    nc.vector.tensor_scalar_mul(ise[:], ise[:], float(TOK + 1))
                    nc.vector.tensor_add(idf[:], idf[:], ise[:])
                    idss16 = isb.tile([16, PERCAP], mybir.dt.int16)
                    nc.vector.tensor_copy(idss16[:], idf[:])
                    nc.sync.dma_start(idx_dram[0:16, 0:PERCAP], idsg16[:])
                    nc.sync.dma_start(idx_dram[16:32, 0:PERCAP], idss16[:])
                    # gate per slot: [16, PERCAP] -> flat [CAP] -> [128, CAP//128]
                    nc.sync.dma_start(
                        gs_dram[:].rearrange("o (i p) -> (o p) i", p=16), gatebuf[:])

                idsg = moe_w.tile([128, PERCAP], mybir.dt.int16)
                idss = moe_w.tile([128, PERCAP], mybir.dt.int16)
                for rep in range(8):
                    nc.sync.dma_start(idsg[rep * 16:(rep + 1) * 16, :], idx_dram[0:16, 0:PERCAP])
                    nc.sync.dma_start(idss[rep * 16:(rep + 1) * 16, :], idx_dram[16:32, 0:PERCAP])
                gslot16 = moe_w.tile([128, CAP // 128], mybir.dt.float16)
                nc.sync.dma_start(gslot16[:], gs_dram[:].rearrange("o (c p) -> (o p) c", p=128))
                gslot = moe_w.tile([128, CAP // 128], f32)
                nc.vector.tensor_copy(gslot[:], gslot16[:])

                with tc.tile_pool(name="moe_sb", bufs=1) as msb, \
                     tc.tile_pool(name="moe_ps", bufs=1, space="PSUM") as mps:
                    NCH = CAP // 512                       # 3 slot chunks of 512
                    for cc in range(NCH):
                        xgT = msb.tile([128, KT, 512], bf16, tag="xgt", bufs=1)
                        nc.gpsimd.dma_gather(
                            out_ap=xgT[:], in_ap=agx_out[:],
                            idxs_ap=idsg[:, cc * 32:(cc + 1) * 32],
                            num_idxs=512, num_idxs_reg=512, elem_size=D, transpose=True)
                        h_sb = msb.tile([128, F // 128, 512], bf16, tag="hsb")
                        for fm in range(F // 128):
                            h_ps = mps.tile([128, 512], f32, space="PSUM", tag="hps", bufs=3)
                            for kt in range(KT):
                                nc.tensor.matmul(h_ps[:], w1b[:, kt, fm * 128:(fm + 1) * 128],
                                                 xgT[:, kt, :], start=(kt == 0), stop=(kt == KT - 1))
                            nc.scalar.activation(h_sb[:, fm, :], h_ps[:],
                                                 mybir.ActivationFunctionType.Gelu)
                        eo_sb = msb.tile([128, 4, D], bf16, tag="eosb", bufs=1)
                        for sl in range(4):
                            for nch in range(2):
                                eo_ps = mps.tile([128, 512], f32, space="PSUM", tag="eops", bufs=3)
                                for ft in range(F // 128):
                                    nc.tensor.matmul(eo_ps[:], h_sb[:, ft, sl * 128:(sl + 1) * 128],
                                                     w2b[:, ft, nch * 512:(nch + 1) * 512],
                                                     start=(ft == 0), stop=(ft == F // 128 - 1))
                                nc.vector.tensor_scalar_mul(
                                    eo_sb[:, sl, nch * 512:(nch + 1) * 512], eo_ps[:],
                                    gslot[:, cc * 4 + sl:cc * 4 + sl + 1])
                        nc.gpsimd.dma_scatter_add(
                            out_ap=partial[:], in_ap=eo_sb[:],
                            idxs_ap=idss[:, cc * 32:(cc + 1) * 32],
                            num_idxs=512, num_idxs_reg=512, elem_size=D)
                moe_w_cm.__exit__(None, None, None)
                w1p_cm.__exit__(None, None, None)
                nc.gpsimd.collective_compute(
                    "ReduceScatter", mybir.AluOpType.add, replica_groups=RG,
                    ins=[partial[0:TOK, :].opt()], outs=[rs2_out[:].opt()])

                if debug:
                    nc.sync.dma_start(dbg["d_selg"][:], agp_out[:])
                    nc.sync.dma_start(dbg["d_ids"][:], idx_dram[:])
                    nc.sync.dma_start(dbg["d_gs"][:], gs_dram[:])
                    nc.sync.dma_start(dbg["d_ns"][:], rs2_out[:])
                # ================= P6: final residual + output ==============
                with tc.tile_pool(name="p6sb", bufs=2) as p6sb:
                    for tt in range(4):
                        ns_t = p6sb.tile([128, D], bf16, tag="nst")
                        nc.sync.dma_start(ns_t[:], rs2_out[tt * 128:(tt + 1) * 128, :])
                        x2_t = p6sb.tile([128, D], f32, tag="x2t")
                        nc.sync.dma_start(x2_t[:], x2f_dram[tt * 128:(tt + 1) * 128, :])
                        o_t = p6sb.tile([128, D], f32, tag="ot")
                        nc.vector.tensor_add(o_t[:], x2_t[:], ns_t[:])
                        nc.sync.dma_start(out_p[tt * 128:(tt + 1) * 128, :], o_t[:])

    nc.compile()
    return nc


def make_in_maps(inputs):
    x = np.asarray(inputs["x"], dtype=np.float32)
    ln1_w = np.asarray(inputs["ln1_w"], dtype=np.float32)
    wqkv = np.asarray(inputs["wqkv"], dtype=np.float32)
    wproj = np.asarray(inputs["wproj"], dtype=np.float32)
    ln2_w = np.asarray(inputs["ln2_w"], dtype=np.float32)
    router_w = np.asarray(inputs["router_w"], dtype=np.float32)
    w1 = np.asarray(inputs["w1"], dtype=np.float32)
    w2 = np.asarray(inputs["w2"], dtype=np.float32)

    x_flat = x.reshape(TOK, D)
    wq_full, wk_full, wv_full = wqkv[:, :D], wqkv[:, D:2 * D], wqkv[:, 2 * D:]

    ident = np.eye(128, dtype=np.float32)
    ones = np.ones((128, 128), dtype=np.float32)
    # causal mask for diagonal 256-unit: [sub*128+p, kk]: 0 if kk <= sub*128+p else -1e9
    causal = np.full((256, 256), -1e9, dtype=np.float32)  # [s*128+p, qq]: 0 if qq >= s*128+p
    for p in range(256):
        causal[p, p:] = 0.0
    ln1_t = ln1_w.reshape(D // 128, 128).T.copy()   # [p, i]
    ln2_t = ln2_w.reshape(D // 128, 128).T.copy()

    in_maps = []
    for c in range(NC):
        rows = slice(c * OWN, (c + 1) * OWN)
        hcols = slice(c * HPC * HD, (c + 1) * HPC * HD)
        esel = np.zeros((128, E), dtype=np.float32)
        esel[:, c] = 1.0
        tokp1 = (np.arange(16)[:, None] * 256 + np.arange(256)[None, :] + 1).astype(np.int16)
        in_maps.append({
            "xT": np.ascontiguousarray(x_flat[rows].T),
            "wq": np.ascontiguousarray(wq_full[:, hcols]),
            "wk": np.ascontiguousarray(wk_full[:, hcols]),
            "wv": np.ascontiguousarray(wv_full[:, hcols]),
            "wproj": wproj,
            "router_w": router_w,
            "ln1_w": ln1_t,
            "ln2_w": ln2_t,
            "w1": w1[c],
            "w2": w2[c],
            "ident": ident,
            "ones": ones,
            "causal": causal,
            "esel": esel,
            "tokp1": tokp1,
        })
    return in_maps


_NC_CACHE = {}


def run(inputs, debug=False, trace=False):
    key = bool(debug)
    if key not in _NC_CACHE:
        _NC_CACHE[key] = build_nc(debug=debug)
    nc = _NC_CACHE[key]
    in_maps = make_in_maps(inputs)
    res = bass_utils.run_bass_kernel_spmd(nc, in_maps, core_ids=list(range(NC)), trace=trace)
    out = np.empty((TOK, D), dtype=np.float32)
    for c in range(NC):
        out[c * OWN:(c + 1) * OWN] = res.results[c]["out"]
    return out.reshape(B, T, D), res


def kernel(**inputs) -> np.ndarray:
    out, _ = run(inputs, debug=False, trace=False)
    return out


# revision 53
# speedup vs baseline: 1.0114x; 1.0114x over previous
"""Distributed Trainium2 Bass kernel for nn_BlockMoE (B=2,T=2048,D=1024,H=16,E=8,K=2).

Sharding (SPMD, one shared instruction stream; all per-core variation via input shards):
  - LN1/LN2/router/output: token-sharded (core c owns global tokens [512c, 512c+512))
  - attention: head-sharded (core c owns heads {2c, 2c+1} via wq/wk/wv column shards)
  - MoE: expert-sharded (core c owns expert c), dense-equivalent compute with gate masking
Collectives: AG(xln1T f32r) -> RS(xoT partials f32r) -> AG(xln2 bf16) + AG(probs f32)
             -> RS(MoE partials bf16).
Attention chain in float32r (TF32-like, full TensorE rate) to keep router top-2
selection faithful; expert MLP in bf16.
"""
import os
import sys
import types

import numpy as np

sys.path.insert(0, '/opt/trn_rl_repo')
sys.path.insert(0, '/opt/trn_rl_repo/concourse')

import concourse.bacc as bacc
import concourse.bass as bass
import concourse.mybir as mybir
import concourse.tile as tile
from concourse import bass_utils

# ---------------------------------------------------------------- trace shim
# bass_utils under BASS_TRACE imports antenv.axon_hooks, absent in this image.
try:
    import antenv
    if not hasattr(antenv, 'axon_hooks'):
        m = types.ModuleType('antenv.axon_hooks')
        m._hook = None
        m.set_axon_ntff_profile_hook = lambda h: setattr(m, '_hook', h)
        m.get_axon_ntff_profile_hook = lambda: m._hook
        sys.modules['antenv.axon_hooks'] = m
        antenv.axon_hooks = m
    if os.environ.get('BASS_TRACE'):
        from antenv.axon_hooks import get_axon_ntff_profile_hook, set_axon_ntff_profile_hook
        if get_axon_ntff_profile_hook() is None:
            from trn_agent_boot.trn_boot import _ntff_profile_via_ctypes
            set_axon_ntff_profile_hook(_ntff_profile_via_ctypes('/opt/axon/libaxon_pjrt.so'))
except Exception:
    pass

B, T, D, H, E, TOPK = 2, 2048, 1024, 16, 8, 2
F = 4 * D
HD = D // H          # 64
NC = 8               # cores
TOK = B * T          # 4096
OWN = TOK // NC      # 512 tokens per core
HPC = H // NC        # 2 heads per core
EPS = 1e-5

f32 = mybir.dt.float32
f32r = mybir.dt.float32r
bf16 = mybir.dt.bfloat16

RG = [list(range(NC))]


def build_nc(debug=False):
    nc = bacc.Bacc("TRN2", num_devices=NC)

    # ---------------- parameters (per-core shards prepared by host) ----------
    xT_p = nc.dram_tensor("xT", [D, OWN], f32r, kind="ExternalInput")          # own tokens, transposed
    wq_p = nc.dram_tensor("wq", [D, HPC * HD], f32r, kind="ExternalInput")     # own heads' q cols
    wk_p = nc.dram_tensor("wk", [D, HPC * HD], f32r, kind="ExternalInput")
    wv_p = nc.dram_tensor("wv", [D, HPC * HD], f32r, kind="ExternalInput")
    wproj_p = nc.dram_tensor("wproj", [D, D], f32r, kind="ExternalInput")  # full (replicated)
    router_p = nc.dram_tensor("router_w", [D, E], f32r, kind="ExternalInput")
    ln1_p = nc.dram_tensor("ln1_w", [128, D // 128], f32, kind="ExternalInput")   # [p, i] = w[i*128+p]
    ln2_p = nc.dram_tensor("ln2_w", [128, D // 128], f32, kind="ExternalInput")
    w1_p = nc.dram_tensor("w1", [D, F], f32, kind="ExternalInput")             # own expert
    w2_p = nc.dram_tensor("w2", [F, D], f32, kind="ExternalInput")
    ident_p = nc.dram_tensor("ident", [128, 128], f32r, kind="ExternalInput")
    ones_p = nc.dram_tensor("ones", [128, 128], f32r, kind="ExternalInput")
    causal_p = nc.dram_tensor("causal", [2 * 128, 256], f32, kind="ExternalInput")  # [sub*128+p, kk]
    esel_p = nc.dram_tensor("esel", [128, E], f32, kind="ExternalInput")       # one-hot row c, replicated
    tokp1_p = nc.dram_tensor("tokp1", [16, 256], mybir.dt.int16, kind="ExternalInput")  # token id + 1

    out_p = nc.dram_tensor("out", [OWN, D], f32, kind="ExternalOutput")
    dbg = {}
    if debug:
        for name, shape, dt_ in [
            ("d_xln1T", [D, OWN], f32), ("d_q", [128, 8 * 512], f32), ("d_k", [128, 8 * 512], f32),
            ("d_v", [128, 32 * 132], f32), ("d_oT", [128, 8 * 512], f32), ("d_xoT", [D, OWN], f32),
            ("d_xln2T", [D, OWN], f32), ("d_probs", [OWN, E], f32), ("d_rsum", [128, 64], f32),
            ("d_attnT", [128, 16 * 512], f32), ("d_selg", [TOK, 2], f32),
            ("d_ids", [32, 128], mybir.dt.int16), ("d_gs", [1, 1536], mybir.dt.float16),
            ("d_ns", [OWN, D], bf16),
        ]:
            dbg[name] = nc.dram_tensor(name, shape, dt_, kind="ExternalOutput")

    KT = D // 128  # 8 contraction tiles over D

    with tile.TileContext(nc) as tc:
        # ---------------- DRAM bounce buffers ------------------------------
        with tc.tile_pool(name="dram", bufs=1, space="DRAM") as dram:
            ag1_in = dram.tile([D, OWN], f32r)                    # xln1T contribution
            ag1_out = dram.tile([NC * D, OWN], f32r, addr_space="Shared")
            a2ao_in = dram.tile([NC * 128, OWN], f32r)            # my heads' oT per owner block
            a2ao_out = dram.tile([NC * 128, OWN], f32r)           # full oT for my tokens
            agx_in = dram.tile([OWN, D], bf16)                    # xln2 rows bf16
            agx_out = dram.tile([TOK, D], bf16, addr_space="Shared")
            agp_in = dram.tile([TOK, 2], f32)                     # own toks x all experts [sel, gate]
            agp_out = dram.tile([TOK, 2], f32)
            idx_dram = dram.tile([32, 128], mybir.dt.int16)       # ids bounce (g in 0:16, s in 16:32)
            gs_dram = dram.tile([1, 1536], mybir.dt.float16)      # gate-per-slot bounce
            partial = dram.tile([TOK + 128, D], bf16)             # scatter table (+trash rows)
            rs2_out = dram.tile([OWN, D], bf16)
            x2f_dram = dram.tile([OWN, D], f32)                   # LN2 rows f32 (for P6)
            rt_dram = dram.tile([16, 128], f32r)                  # recip flatten bounce
            gt_dram = dram.tile([4, 128], f32r)                   # gate flatten bounce

            # ---------------- persistent SBUF ------------------------------
            with tc.tile_pool(name="persist", bufs=1) as pp:
                ident = pp.tile([128, 128], f32r)
                nc.sync.dma_start(ident[:], ident_p[:])
                ident_bf = pp.tile([128, 128], bf16)
                nc.vector.tensor_copy(ident_bf[:], ident[:])
                ones = pp.tile([128, 128], f32r)
                nc.sync.dma_start(ones[:], ones_p[:])
                causal = pp.tile([128, 2, 256], f32)
                nc.sync.dma_start(causal[:], causal_p[:].rearrange("(s p) k -> p s k", p=128))
                ln2w = pp.tile([128, KT], f32)
                nc.sync.dma_start(ln2w[:], ln2_p[:])
                esel = pp.tile([128, E], f32)
                nc.sync.dma_start(esel[:], esel_p[:])
                xln2 = pp.tile([128, KT, OWN], f32r)              # LN2 output (own)

                pa_cm = tc.tile_pool(name="phaseA", bufs=1)
                pa = pa_cm.__enter__()

                # ---------- helper: layernorm in [feat, tok] layout ----------
                def layer_norm_T(src, dst, wcol, psum_pool, sbuf_pool):
                    """src, dst: [128, KT, OWN] (f32-readable); wcol [128, KT]."""
                    sum_ps = psum_pool.tile([1, OWN], f32, space="PSUM")
                    sq_ps = psum_pool.tile([1, OWN], f32, space="PSUM")
                    for kt in range(KT):
                        nc.tensor.matmul(sum_ps[:], ones[:, :1], src[:, kt, :],
                                         start=(kt == 0), stop=(kt == KT - 1))
                    for kt in range(KT):
                        sqt = sbuf_pool.tile([128, OWN], f32r, tag="lnsq", bufs=1)
                        nc.vector.tensor_tensor(out=sqt[:], in0=src[:, kt, :], in1=src[:, kt, :],
                                                op=mybir.AluOpType.mult)
                        nc.tensor.matmul(sq_ps[:], ones[:, :1], sqt[:],
                                         start=(kt == 0), stop=(kt == KT - 1))
                    mu = sbuf_pool.tile([1, OWN], f32, tag="lnmu")
                    nc.vector.tensor_scalar_mul(mu[:], sum_ps[:], 1.0 / D)
                    msq = sbuf_pool.tile([1, OWN], f32, tag="lnmsq")
                    nc.vector.tensor_scalar_mul(msq[:], sq_ps[:], 1.0 / D)
                    mu2 = sbuf_pool.tile([1, OWN], f32, tag="lnmu2")
                    nc.vector.tensor_tensor(out=mu2[:], in0=mu[:], in1=mu[:], op=mybir.AluOpType.mult)
                    var = sbuf_pool.tile([1, OWN], f32, tag="lnvar")
                    nc.vector.tensor_sub(var[:], msq[:], mu2[:])
                    nc.vector.tensor_scalar_add(var[:], var[:], EPS)
                    std = sbuf_pool.tile([1, OWN], f32, tag="lnstd")
                    nc.scalar.activation(std[:], var[:], mybir.ActivationFunctionType.Sqrt)
                    rstd = sbuf_pool.tile([1, OWN], f32, tag="lnrstd")
                    nc.vector.reciprocal(rstd[:], std[:])
                    mur = sbuf_pool.tile([1, OWN], f32r, tag="lnmur")
                    nc.vector.tensor_copy(mur[:], mu[:])
                    rstdr = sbuf_pool.tile([1, OWN], f32r, tag="lnrstdr")
                    nc.vector.tensor_copy(rstdr[:], rstd[:])
                    mu_b = psum_pool.tile([128, OWN], f32, space="PSUM")
                    rstd_b = psum_pool.tile([128, OWN], f32, space="PSUM")
                    nc.tensor.matmul(mu_b[:], ones[:1, :], mur[:], start=True, stop=True)
                    nc.tensor.matmul(rstd_b[:], ones[:1, :], rstdr[:], start=True, stop=True)
                    for kt in range(KT):
                        t1 = sbuf_pool.tile([128, OWN], f32, tag="lnt1")
                        nc.vector.tensor_sub(t1[:], src[:, kt, :], mu_b[:])
                        t2 = sbuf_pool.tile([128, OWN], f32, tag="lnt2")
                        nc.vector.tensor_tensor(out=t2[:], in0=t1[:], in1=rstd_b[:], op=mybir.AluOpType.mult)
                        nc.vector.tensor_scalar_mul(dst[:, kt, :], t2[:], wcol[:, kt:kt + 1])

                # ================= P0: LN1 + AG ===========================
                with tc.tile_pool(name="p0sb", bufs=1) as p0sb, \
                     tc.tile_pool(name="p0ps", bufs=1, space="PSUM") as p0ps:
                    ln1w = p0sb.tile([128, KT], f32)
                    nc.sync.dma_start(ln1w[:], ln1_p[:])
                    xt = p0sb.tile([128, KT, OWN], f32r)          # own xT
                    nc.sync.dma_start(xt[:], xT_p[:].rearrange("(kt p) t -> p kt t", p=128))
                    xln1 = p0sb.tile([128, KT, OWN], f32r)
                    layer_norm_T(xt, xln1, ln1w, p0ps, p0sb)
                    nc.sync.dma_start(ag1_in[:].rearrange("(kt p) t -> p kt t", p=128), xln1[:])
                    if debug:
                        nc.sync.dma_start(dbg["d_xln1T"][:].rearrange("(kt p) t -> p kt t", p=128), xln1[:].bitcast(f32))
                nc.gpsimd.collective_compute(
                    "AllGather", mybir.AluOpType.bypass, replica_groups=RG,
                    ins=[ag1_in[:].opt()], outs=[ag1_out[:].opt()])
                # zero the MoE scatter table (runs early, overlaps attention)
                with tc.tile_pool(name="zpool", bufs=1) as zp:
                    zt = zp.tile([128, D], bf16)
                    nc.vector.memset(zt[:], 0)
                    for zi in range((TOK + 128) // 128):
                        nc.sync.dma_start(partial[zi * 128:(zi + 1) * 128, :], zt[:])

                # ================= P1: qkv (own 2 heads, all tokens) =======

                with tc.tile_pool(name="attn_sb", bufs=1) as asb:
                    p1ps_cm = tc.tile_pool(name="p1ps", bufs=1, space="PSUM")
                    aps = p1ps_cm.__enter__()
                    tps = aps
                    wqp_cm = tc.tile_pool(name="wqp", bufs=1)
                    wqp = wqp_cm.__enter__()
                    wq = wqp.tile([128, KT, HPC * HD], f32r)
                    nc.sync.dma_start(wq[:], wq_p[:].rearrange("(kt p) m -> p kt m", p=128))
                    wk = wqp.tile([128, KT, HPC * HD], f32r)
                    nc.sync.dma_start(wk[:], wk_p[:].rearrange("(kt p) m -> p kt m", p=128))
                    wv = wqp.tile([128, KT, HPC * HD], f32r)
                    nc.sync.dma_start(wv[:], wv_p[:].rearrange("(kt p) m -> p kt m", p=128))
                    q_sb = asb.tile([128, NC, 512], f32r)   # [2h*64, rblk, tok]
                    k_sb = asb.tile([128, NC, 512], f32r)
                    v_sb = asb.tile([128, 32, 132], f32r)   # [tok128, t-tile, h*65+{64 feat, 1 ones}]
                    for _t in range(32):
                        nc.vector.tensor_copy(v_sb[:, _t, 64:65], ones[:, :1])
                        nc.vector.tensor_copy(v_sb[:, _t, 129:130], ones[:, :1])
                    for r in range(NC):
                        xg1_r = wqp.tile([128, KT, 512], f32r, tag="xg1", bufs=2)
                        nc.sync.dma_start(
                            xg1_r[:], ag1_out[r * D:(r + 1) * D, :].rearrange("(kt p) t -> p kt t", p=128))
                        q_ps = aps.tile([128, 512], f32, space="PSUM", tag="qkv", bufs=3)
                        for kt in range(KT):
                            nc.tensor.matmul(q_ps[:HPC * HD, :], wq[:, kt, :], xg1_r[:, kt, :],
                                             start=(kt == 0), stop=(kt == KT - 1))
                        nc.vector.tensor_copy(q_sb[:HPC * HD, r, :], q_ps[:HPC * HD, :])
                        k_ps = aps.tile([128, 512], f32, space="PSUM", tag="qkv", bufs=3)
                        for kt in range(KT):
                            nc.tensor.matmul(k_ps[:HPC * HD, :], wk[:, kt, :], xg1_r[:, kt, :],
                                             start=(kt == 0), stop=(kt == KT - 1))
                        nc.vector.tensor_copy(k_sb[:HPC * HD, r, :], k_ps[:HPC * HD, :])
                        v_ps = aps.tile([128, 512], f32, space="PSUM", tag="qkv", bufs=3)
                        for kt in range(KT):
                            nc.tensor.matmul(v_ps[:HPC * HD, :], wv[:, kt, :], xg1_r[:, kt, :],
                                             start=(kt == 0), stop=(kt == KT - 1))
                        vT_sb = asb.tile([128, 512], f32r, tag="vT", bufs=2)
                        nc.vector.tensor_copy(vT_sb[:HPC * HD, :], v_ps[:HPC * HD, :])
                        # transpose v to [tok, feat]; interleave ones col per head
                        for tt in range(4):
                            v_tps = tps.tile([128, 128], f32r, space="PSUM", tag="vtr", bufs=3)
                            nc.tensor.transpose(v_tps[:], vT_sb[:, tt * 128:(tt + 1) * 128], ident[:])
                            nc.vector.tensor_copy(v_sb[:, r * 4 + tt, 0:64], v_tps[:, 0:64])
                            nc.vector.tensor_copy(v_sb[:, r * 4 + tt, 65:129], v_tps[:, 64:128])
                    if debug:
                        nc.sync.dma_start(dbg["d_q"][:].rearrange("p (r t) -> p r t", r=NC), q_sb[:].bitcast(f32))
                        nc.sync.dma_start(dbg["d_k"][:].rearrange("p (r t) -> p r t", r=NC), k_sb[:].bitcast(f32))
                        nc.sync.dma_start(dbg["d_v"][:].rearrange("p (r t) -> p r t", r=32, t=132), v_sb[:].bitcast(f32))

                    wqp_cm.__exit__(None, None, None)
                    p1ps_cm.__exit__(None, None, None)
                    p2ps_cm = tc.tile_pool(name="p2ps", bufs=1, space="PSUM")
                    aps = p2ps_cm.__enter__()
                    tps = aps
                    # ============= P2: scores/softmax/AV per (b, h) =========
                    oT_sb = asb.tile([128, NC, 512], f32r)   # [2h*64, rblk, tok]
                    for b in range(B):
                        for h in range(HPC):
                            hs = h * HD
                            PT = asb.tile([128, 16, 512], f32r, tag="attnT", bufs=1)
                            for qc in range(4):
                                rq = b * 4 + qc
                                nkt = 4 * qc + 4
                                for kt in range(nkt):
                                    u = kt // 2
                                    ru = b * 4 + u // 2
                                    ik = (u % 2) * 256 + (kt % 2) * 128
                                    qs = max(0, u * 256 - qc * 512)
                                    s_ps = aps.tile([128, 512], f32, space="PSUM", tag="score", bufs=3)
                                    nc.tensor.matmul(s_ps[:, qs:512],
                                                     k_sb[hs:hs + HD, ru, ik:ik + 128],
                                                     q_sb[hs:hs + HD, rq, qs:512],
                                                     start=True, stop=True)
                                    dq = u * 256 - qc * 512   # diag block q-col start
                                    if 0 <= dq < 512:
                                        nc.vector.tensor_add(s_ps[:, dq:dq + 256], s_ps[:, dq:dq + 256],
                                                             causal[:, kt % 2, :])
                                    nc.scalar.activation(PT[:, kt, qs:512], s_ps[:, qs:512],
                                                         mybir.ActivationFunctionType.Exp, scale=0.125)
                                o_ps = aps.tile([128, 512], f32, space="PSUM", tag="avps", bufs=3)
                                for kt in range(nkt):
                                    qs = max(0, (kt // 2) * 256 - qc * 512)
                                    nc.tensor.matmul(
                                        o_ps[:HD + 1, qs:512],
                                        v_sb[:, b * 16 + kt, h * 65:h * 65 + 65],
                                        PT[:, kt, qs:512],
                                        start=(kt == 0), stop=(kt == nkt - 1))
                                rs_row = asb.tile([1, 512], f32, tag="rsrow", bufs=2)
                                nc.vector.reciprocal(rs_row[:], o_ps[HD:HD + 1, :])
                                rcp_row = asb.tile([1, 512], f32r, tag="rcprow", bufs=2)
                                nc.vector.tensor_copy(rcp_row[:], rs_row[:])
                                rb_ps = aps.tile([128, 512], f32, space="PSUM", tag="rbcast", bufs=1)
                                nc.tensor.matmul(rb_ps[:], ones[:1, :], rcp_row[:], start=True, stop=True)
                                rb_sb = asb.tile([128, 512], f32, tag="rbsb", bufs=2)
                                nc.vector.tensor_copy(rb_sb[:], rb_ps[:])
                                nc.vector.tensor_tensor(
                                    out=oT_sb[hs:hs + HD, b * 4 + qc, :],
                                    in0=o_ps[:HD, :], in1=rb_sb[:HD, :], op=mybir.AluOpType.mult)
                    if debug:
                        nc.sync.dma_start(dbg["d_oT"][:].rearrange("p (r t) -> p r t", r=NC), oT_sb[:].bitcast(f32))

                    p2ps_cm.__exit__(None, None, None)
                    # ============= P3: ship oT blocks to token owners =======
                    for r in range(NC):
                        nc.sync.dma_start(a2ao_in[r * 128:(r + 1) * 128, :], oT_sb[:, r, :])
                pa_cm.__exit__(None, None, None)
                PERCAP = 96
                CAP = 16 * PERCAP                                  # 1536 slots
                nc.gpsimd.collective_compute(
                    "AllToAll", mybir.AluOpType.bypass, replica_groups=RG,
                    ins=[a2ao_in[:].opt()], outs=[a2ao_out[:].opt()])

                # ================= P4: residual + LN2 + router ==============
                router_w = pp.tile([128, KT, E], f32r)
                nc.sync.dma_start(router_w[:], router_p[:].rearrange("(kt p) e -> p kt e", p=128))
                with tc.tile_pool(name="p4sb", bufs=1) as p4sb, \
                     tc.tile_pool(name="p4ps", bufs=1, space="PSUM") as p4ps:
                    xres = p4sb.tile([128, KT, OWN], f32r)
                    p4o_cm = tc.tile_pool(name="p4o", bufs=1)
                    p4o = p4o_cm.__enter__()
                    oT_full = p4o.tile([128, KT, OWN], f32r)
                    nc.sync.dma_start(oT_full[:], a2ao_out[:].rearrange("(kt p) t -> p kt t", p=128))
                    for dm in range(KT):
                        pj_ps = p4ps.tile([128, OWN], f32, space="PSUM", tag="proj", bufs=2)
                        for kt in range(KT):
                            wpj_t = p4o.tile([128, 128], f32r, tag="wpjt", bufs=2)
                            nc.sync.dma_start(wpj_t[:], wproj_p[kt * 128:(kt + 1) * 128,
                                                                dm * 128:(dm + 1) * 128])
                            nc.tensor.matmul(pj_ps[:], wpj_t[:], oT_full[:, kt, :],
                                             start=(kt == 0), stop=(kt == KT - 1))
                        xt_t = p4sb.tile([128, OWN], f32r, tag="xtt", bufs=2)
                        nc.sync.dma_start(xt_t[:], xT_p[dm * 128:(dm + 1) * 128, :])
                        nc.vector.tensor_add(xres[:, dm, :], xt_t[:], pj_ps[:])
                    p4o_cm.__exit__(None, None, None)
                    if debug:
                        nc.sync.dma_start(dbg["d_xoT"][:].rearrange("(kt p) t -> p kt t", p=128), xres[:].bitcast(f32))
                    layer_norm_T(xres, xln2, ln2w, p4ps, p4sb)
                    if debug:
                        nc.sync.dma_start(dbg["d_xln2T"][:].rearrange("(kt p) t -> p kt t", p=128), xln2[:].bitcast(f32))
                    # transpose xln2 -> token-row layout (bf16 for gather table, f32 for P6)
                    x2row = p4sb.tile([128, 4, D], bf16)
                    for kt in range(KT):
                        for tt in range(4):
                            x2_tps = p4ps.tile([128, 128], f32r, space="PSUM", tag="x2tr", bufs=1)
                            nc.tensor.transpose(x2_tps[:], xln2[:, kt, tt * 128:(tt + 1) * 128], ident[:])
                            nc.vector.tensor_copy(x2row[:, tt, kt * 128:(kt + 1) * 128], x2_tps[:])
                            x2f_t = p4sb.tile([128, 128], f32, tag="x2ft", bufs=2)
                            nc.vector.tensor_copy(x2f_t[:], x2_tps[:])
                            nc.sync.dma_start(x2f_dram[tt * 128:(tt + 1) * 128, kt * 128:(kt + 1) * 128],
                                              x2f_t[:])
                    nc.sync.dma_start(agx_in[:].rearrange("(tt p) d2 -> p tt d2", p=128), x2row[:])
                    # router: logits [tok, E] for own tokens
                    probs = p4sb.tile([128, 4, E], f32)
                    for mt in range(4):
                        lg_ps = p4ps.tile([128, E], f32, space="PSUM", tag="router", bufs=1)
                        for kt in range(KT):
                            nc.tensor.matmul(lg_ps[:], xln2[:, kt, mt * 128:(mt + 1) * 128],
                                             router_w[:, kt, :], start=(kt == 0), stop=(kt == KT - 1))
                        pex = p4sb.tile([128, E], f32, tag="pex", bufs=2)
                        nc.scalar.activation(pex[:], lg_ps[:], mybir.ActivationFunctionType.Exp)
                        psum_r = p4sb.tile([128, 1], f32, tag="psr", bufs=2)
                        nc.vector.tensor_reduce(psum_r[:], pex[:], axis=mybir.AxisListType.X,
                                                op=mybir.AluOpType.add)
                        prcp = p4sb.tile([128, 1], f32, tag="prcp", bufs=2)
                        nc.vector.reciprocal(prcp[:], psum_r[:])
                        nc.vector.tensor_scalar_mul(probs[:, mt, :], pex[:], prcp[:])
                    # own-token [sel, gate] for EVERY expert, A2A-dispatched
                    selg = p4sb.tile([128, E, 4, 2], f32)
                    for mt in range(4):
                        m8 = p4sb.tile([128, 8], f32, tag="m8", bufs=2)
                        nc.vector.max(out=m8[:], in_=probs[:, mt, :])
                        den = p4sb.tile([128, 1], f32, tag="den", bufs=2)
                        nc.vector.tensor_add(den[:], m8[:, 0:1], m8[:, 1:2])
                        rden = p4sb.tile([128, 1], f32, tag="rden", bufs=2)
                        nc.vector.reciprocal(rden[:], den[:])
                        for e in range(E):
                            pe = probs[:, mt, e:e + 1]
                            nc.vector.tensor_tensor(out=selg[:, e, mt, 0:1], in0=pe, in1=m8[:, 1:2],
                                                    op=mybir.AluOpType.is_ge)
                            g1 = p4sb.tile([128, 1], f32, tag="g1", bufs=2)
                            nc.vector.tensor_tensor(out=g1[:], in0=pe, in1=rden[:],
                                                    op=mybir.AluOpType.mult)
                            nc.vector.tensor_tensor(out=selg[:, e, mt, 1:2], in0=g1[:],
                                                    in1=selg[:, e, mt, 0:1],
                                                    op=mybir.AluOpType.mult)
                    nc.sync.dma_start(agp_in[:].rearrange("(e mt p) o -> p e mt o", p=128, mt=4), selg[:])
                    if debug:
                        nc.sync.dma_start(dbg["d_probs"][:].rearrange("(mt p) e -> p mt e", p=128), probs[:])
                nc.gpsimd.collective_compute(
                    "AllToAll", mybir.AluOpType.bypass, replica_groups=RG,
                    ins=[agp_in[:].opt()], outs=[agp_out[:].opt()])
                nc.gpsimd.collective_compute(
                    "AllGather", mybir.AluOpType.bypass, replica_groups=RG,
                    ins=[agx_in[:].opt()], outs=[agx_out[:].opt()])
                moe_w_cm = tc.tile_pool(name="moe_w", bufs=1)
                moe_w = moe_w_cm.__enter__()
                w1b = moe_w.tile([128, KT, F], bf16)       # [Dpart, kt, F]
                w2b = moe_w.tile([128, F // 128, D], bf16)  # [Fpart, ft, D]
                with tc.tile_pool(name="wconv", bufs=3) as wcp:
                    for kt in range(KT):
                        for ch in range(2):
                            wt = wcp.tile([128, 2048], f32, tag="wc32")
                            nc.sync.dma_start(wt[:], w1_p[kt * 128:(kt + 1) * 128,
                                                          ch * 2048:(ch + 1) * 2048])
                            nc.vector.tensor_copy(w1b[:, kt, ch * 2048:(ch + 1) * 2048], wt[:])
                    for ft in range(F // 128):
                        for ch in range(1):
                            wt = wcp.tile([128, 2048], f32, tag="wc32")
                            nc.sync.dma_start(wt[:, :1024], w2_p[ft * 128:(ft + 1) * 128, :])
                            nc.vector.tensor_copy(w2b[:, ft, :], wt[:, :1024])


                # ================= P5: routed expert (own expert) ===========
                # ---- index build: compact token list for own expert ----
                with tc.tile_pool(name="idx_sb", bufs=1) as isb:
                    selw = isb.tile([16, 256], f32)
                    nc.sync.dma_start(selw[:], agp_out[:, 0:1].rearrange("(p j) o -> p (j o)", p=16))
                    gatew = isb.tile([16, 256], f32)
                    nc.sync.dma_start(gatew[:], agp_out[:, 1:2].rearrange("(p j) o -> p (j o)", p=16))
                    tokp1 = isb.tile([16, 256], mybir.dt.int16)
                    nc.sync.dma_start(tokp1[:], tokp1_p[:])
                    incl = isb.tile([16, 256], f32)
                    nc.vector.tensor_tensor_scan(incl[:], selw[:], selw[:], 0.0,
                                                 op0=mybir.AluOpType.add, op1=mybir.AluOpType.bypass)
                    pos = isb.tile([16, 256], f32)
                    nc.vector.tensor_sub(pos[:], incl[:], selw[:])
                    # pos_m = pos*sel + sel - 1  (-1 for unselected), clamped
                    nc.vector.tensor_tensor(out=pos[:], in0=pos[:], in1=selw[:], op=mybir.AluOpType.mult)
                    nc.vector.tensor_add(pos[:], pos[:], selw[:])
                    nc.vector.tensor_scalar_add(pos[:], pos[:], -1.0)
                    nc.vector.tensor_scalar_min(pos[:], pos[:], float(PERCAP - 1))
                    pos16 = isb.tile([16, 256], mybir.dt.int16)
                    nc.vector.tensor_copy(pos16[:], pos[:])
                    idbuf = isb.tile([16, PERCAP], mybir.dt.int16)
                    nc.gpsimd.local_scatter(idbuf[:], tokp1[:], pos16[:], channels=16,
                                            num_elems=PERCAP, num_idxs=256)
                    gate16 = isb.tile([16, 256], mybir.dt.float16)
                    nc.vector.tensor_copy(gate16[:], gatew[:])
                    gatebuf = isb.tile([16, PERCAP], mybir.dt.float16)
                    nc.gpsimd.local_scatter(gatebuf[:], gate16[:], pos16[:], channels=16,
                                            num_elems=PERCAP, num_idxs=256)
                    # fixups in f32: gather ids = max(id-1, 0); scatter ids = (id==0) ? TOK+p : id-1
                    idf = isb.tile([16, PERCAP], f32)
                    nc.vector.tensor_copy(idf[:], idbuf[:])
                    ise = isb.tile([16, PERCAP], f32)
                    nc.vector.tensor_scalar(ise[:], idf[:], 0.0, scalar2=None,
                                            op0=mybir.AluOpType.is_equal)
                    nc.vector.tensor_scalar_add(idf[:], idf[:], -1.0)
                    gth = isb.tile([16, PERCAP], f32)
                    nc.vector.tensor_scalar_max(gth[:], idf[:], 0.0)
                    idsg16 = isb.tile([16, PERCAP], mybir.dt.int16)
                    nc.vector.tensor_copy(idsg16[:], gth[:])
                    nc.vector.tensor_scalar_mul(ise[:], ise[:], float(TOK + 1))
                    nc.vector.tensor_add(idf[:], idf[:], ise[:])
                    idss16 = isb.tile([16, PERCAP], mybir.dt.int16)
                    nc.vector.tensor_copy(idss16[:], idf[:])
                    nc.sync.dma_start(idx_dram[0:16, 0:PERCAP], idsg16[:])
                    nc.sync.dma_start(idx_dram[16:32, 0:PERCAP], idss16[:])
                    # gate per slot: [16, PERCAP] -> flat [CAP] -> [128, CAP//128]
                    nc.sync.dma_start(
                        gs_dram[:].rearrange("o (i p) -> (o p) i", p=16), gatebuf[:])

                idsg = moe_w.tile([128, PERCAP], mybir.dt.int16)
                idss = moe_w.tile([128, PERCAP], mybir.dt.int16)
                for rep in range(8):
                    nc.sync.dma_start(idsg[rep * 16:(rep + 1) * 16, :], idx_dram[0:16, 0:PERCAP])
                    nc.sync.dma_start(idss[rep * 16:(rep + 1) * 16, :], idx_dram[16:32, 0:PERCAP])
                gslot16 = moe_w.tile([128, CAP // 128], mybir.dt.float16)
                nc.sync.dma_start(gslot16[:], gs_dram[:].rearrange("o (c p) -> (o p) c", p=128))
                gslot = moe_w.tile([128, CAP // 128], f32)
                nc.vector.tensor_copy(gslot[:], gslot16[:])

                with tc.tile_pool(name="moe_sb", bufs=1) as msb, \
                     tc.tile_pool(name="moe_ps", bufs=1, space="PSUM") as mps:
                    NCH = CAP // 512                       # 3 slot chunks of 512
                    for cc in range(NCH):
                        xgT = msb.tile([128, KT, 512], bf16, tag="xgt", bufs=2)
                        nc.gpsimd.dma_gather(
                            out_ap=xgT[:], in_ap=agx_out[:],
                            idxs_ap=idsg[:, cc * 32:(cc + 1) * 32],
                            num_idxs=512, num_idxs_reg=512, elem_size=D, transpose=True)
                        h_sb = msb.tile([128, F // 128, 512], bf16, tag="hsb")
                        for fm in range(F // 128):
                            h_ps = mps.tile([128, 512], f32, space="PSUM", tag="hps", bufs=3)
                            for kt in range(KT):
                                nc.tensor.matmul(h_ps[:], w1b[:, kt, fm * 128:(fm + 1) * 128],
                                                 xgT[:, kt, :], start=(kt == 0), stop=(kt == KT - 1))
                            nc.scalar.activation(h_sb[:, fm, :], h_ps[:],
                                                 mybir.ActivationFunctionType.Gelu)
                        eo_sb = msb.tile([128, 4, D], bf16, tag="eosb", bufs=1)
                        for sl in range(4):
                            for nch in range(2):
                                eo_ps = mps.tile([128, 512], f32, space="PSUM", tag="eops", bufs=3)
                                for ft in range(F // 128):
                                    nc.tensor.matmul(eo_ps[:], h_sb[:, ft, sl * 128:(sl + 1) * 128],
                                                     w2b[:, ft, nch * 512:(nch + 1) * 512],
                                                     start=(ft == 0), stop=(ft == F // 128 - 1))
                                nc.vector.tensor_scalar_mul(
                                    eo_sb[:, sl, nch * 512:(nch + 1) * 512], eo_ps[:],
                                    gslot[:, cc * 4 + sl:cc * 4 + sl + 1])
                        nc.gpsimd.dma_scatter_add(
                            out_ap=partial[:], in_ap=eo_sb[:],
                            idxs_ap=idss[:, cc * 32:(cc + 1) * 32],
                            num_idxs=512, num_idxs_reg=512, elem_size=D)
                moe_w_cm.__exit__(None, None, None)
                nc.gpsimd.collective_compute(
                    "ReduceScatter", mybir.AluOpType.add, replica_groups=RG,
                    ins=[partial[0:TOK, :].opt()], outs=[rs2_out[:].opt()])

                if debug:
                    nc.sync.dma_start(dbg["d_selg"][:], agp_out[:])
                    nc.sync.dma_start(dbg["d_ids"][:], idx_dram[:])
                    nc.sync.dma_start(dbg["d_gs"][:], gs_dram[:])
                    nc.sync.dma_start(dbg["d_ns"][:], rs2_out[:])
                # ================= P6: final residual + output ==============
                with tc.tile_pool(name="p6sb", bufs=2) as p6sb:
                    for tt in range(4):
                        ns_t = p6sb.tile([128, D], bf16, tag="nst")
                        nc.sync.dma_start(ns_t[:], rs2_out[tt * 128:(tt + 1) * 128, :])
                        x2_t = p6sb.tile([128, D], f32, tag="x2t")
                        nc.sync.dma_start(x2_t[:], x2f_dram[tt * 128:(tt + 1) * 128, :])
                        o_t = p6sb.tile([128, D], f32, tag="ot")
                        nc.vector.tensor_add(o_t[:], x2_t[:], ns_t[:])
                        nc.sync.dma_start(out_p[tt * 128:(tt + 1) * 128, :], o_t[:])

    nc.compile()
    return nc


def make_in_maps(inputs):
    x = np.asarray(inputs["x"], dtype=np.float32)
    ln1_w = np.asarray(inputs["ln1_w"], dtype=np.float32)
    wqkv = np.asarray(inputs["wqkv"], dtype=np.float32)
    wproj = np.asarray(inputs["wproj"], dtype=np.float32)
    ln2_w = np.asarray(inputs["ln2_w"], dtype=np.float32)
    router_w = np.asarray(inputs["router_w"], dtype=np.float32)
    w1 = np.asarray(inputs["w1"], dtype=np.float32)
    w2 = np.asarray(inputs["w2"], dtype=np.float32)

    x_flat = x.reshape(TOK, D)
    wq_full, wk_full, wv_full = wqkv[:, :D], wqkv[:, D:2 * D], wqkv[:, 2 * D:]

    ident = np.eye(128, dtype=np.float32)
    ones = np.ones((128, 128), dtype=np.float32)
    # causal mask for diagonal 256-unit: [sub*128+p, kk]: 0 if kk <= sub*128+p else -1e9
    causal = np.full((256, 256), -1e9, dtype=np.float32)  # [s*128+p, qq]: 0 if qq >= s*128+p
    for p in range(256):
        causal[p, p:] = 0.0
    ln1_t = ln1_w.reshape(D // 128, 128).T.copy()   # [p, i]
    ln2_t = ln2_w.reshape(D // 128, 128).T.copy()

    in_maps = []
    for c in range(NC):
        rows = slice(c * OWN, (c + 1) * OWN)
        hcols = slice(c * HPC * HD, (c + 1) * HPC * HD)
        esel = np.zeros((128, E), dtype=np.float32)
        esel[:, c] = 1.0
        tokp1 = (np.arange(16)[:, None] * 256 + np.arange(256)[None, :] + 1).astype(np.int16)
        in_maps.append({
            "xT": np.ascontiguousarray(x_flat[rows].T),
            "wq": np.ascontiguousarray(wq_full[:, hcols]),
            "wk": np.ascontiguousarray(wk_full[:, hcols]),
            "wv": np.ascontiguousarray(wv_full[:, hcols]),
            "wproj": wproj,
            "router_w": router_w,
            "ln1_w": ln1_t,
            "ln2_w": ln2_t,
            "w1": w1[c],
            "w2": w2[c],
            "ident": ident,
            "ones": ones,
            "causal": causal,
            "esel": esel,
            "tokp1": tokp1,
        })
    return in_maps


_NC_CACHE = {}


def run(inputs, debug=False, trace=False):
    key = bool(debug)
    if key not in _NC_CACHE:
        _NC_CACHE[key] = build_nc(debug=debug)
    nc = _NC_CACHE[key]
    in_maps = make_in_maps(inputs)
    res = bass_utils.run_bass_kernel_spmd(nc, in_maps, core_ids=list(range(NC)), trace=trace)
    out = np.empty((TOK, D), dtype=np.float32)
    for c in range(NC):
        out[c * OWN:(c + 1) * OWN] = res.results[c]["out"]
    return out.reshape(B, T, D), res


def kernel(**inputs) -> np.ndarray:
    out, _ = run(inputs, debug=False, trace=False)
    return out


# revision 54
# speedup vs baseline: 1.0673x; 1.0553x over previous
"""Distributed Trainium2 Bass kernel for nn_BlockMoE (B=2,T=2048,D=1024,H=16,E=8,K=2).

Sharding (SPMD, one shared instruction stream; all per-core variation via input shards):
  - LN1/LN2/router/output: token-sharded (core c owns global tokens [512c, 512c+512))
  - attention: head-sharded (core c owns heads {2c, 2c+1} via wq/wk/wv column shards)
  - MoE: expert-sharded (core c owns expert c), dense-equivalent compute with gate masking
Collectives: AG(xln1T f32r) -> RS(xoT partials f32r) -> AG(xln2 bf16) + AG(probs f32)
             -> RS(MoE partials bf16).
Attention chain in float32r (TF32-like, full TensorE rate) to keep router top-2
selection faithful; expert MLP in bf16.
"""
import os
import sys
import types

import numpy as np

sys.path.insert(0, '/opt/trn_rl_repo')
sys.path.insert(0, '/opt/trn_rl_repo/concourse')

import concourse.bacc as bacc
import concourse.bass as bass
import concourse.mybir as mybir
import concourse.tile as tile
from concourse import bass_utils

# ---------------------------------------------------------------- trace shim
# bass_utils under BASS_TRACE imports antenv.axon_hooks, absent in this image.
try:
    import antenv
    if not hasattr(antenv, 'axon_hooks'):
        m = types.ModuleType('antenv.axon_hooks')
        m._hook = None
        m.set_axon_ntff_profile_hook = lambda h: setattr(m, '_hook', h)
        m.get_axon_ntff_profile_hook = lambda: m._hook
        sys.modules['antenv.axon_hooks'] = m
        antenv.axon_hooks = m
    if os.environ.get('BASS_TRACE'):
        from antenv.axon_hooks import get_axon_ntff_profile_hook, set_axon_ntff_profile_hook
        if get_axon_ntff_profile_hook() is None:
            from trn_agent_boot.trn_boot import _ntff_profile_via_ctypes
            set_axon_ntff_profile_hook(_ntff_profile_via_ctypes('/opt/axon/libaxon_pjrt.so'))
except Exception:
    pass

B, T, D, H, E, TOPK = 2, 2048, 1024, 16, 8, 2
F = 4 * D
HD = D // H          # 64
NC = 8               # cores
TOK = B * T          # 4096
OWN = TOK // NC      # 512 tokens per core
HPC = H // NC        # 2 heads per core
EPS = 1e-5

f32 = mybir.dt.float32
f32r = mybir.dt.float32r
bf16 = mybir.dt.bfloat16

RG = [list(range(NC))]


def build_nc(debug=False):
    nc = bacc.Bacc("TRN2", num_devices=NC)

    # ---------------- parameters (per-core shards prepared by host) ----------
    xT_p = nc.dram_tensor("xT", [D, OWN], f32r, kind="ExternalInput")          # own tokens, transposed
    wq_p = nc.dram_tensor("wq", [D, HPC * HD], f32r, kind="ExternalInput")     # own heads' q cols
    wk_p = nc.dram_tensor("wk", [D, HPC * HD], f32r, kind="ExternalInput")
    wv_p = nc.dram_tensor("wv", [D, HPC * HD], f32r, kind="ExternalInput")
    wproj_p = nc.dram_tensor("wproj", [D, D], f32r, kind="ExternalInput")  # full (replicated)
    router_p = nc.dram_tensor("router_w", [D, E], f32r, kind="ExternalInput")
    ln1_p = nc.dram_tensor("ln1_w", [128, D // 128], f32, kind="ExternalInput")   # [p, i] = w[i*128+p]
    ln2_p = nc.dram_tensor("ln2_w", [128, D // 128], f32, kind="ExternalInput")
    w1_p = nc.dram_tensor("w1", [D, F], f32, kind="ExternalInput")             # own expert
    w2_p = nc.dram_tensor("w2", [F, D], f32, kind="ExternalInput")
    ident_p = nc.dram_tensor("ident", [128, 128], f32r, kind="ExternalInput")
    ones_p = nc.dram_tensor("ones", [128, 128], f32r, kind="ExternalInput")
    causal_p = nc.dram_tensor("causal", [2 * 128, 256], f32, kind="ExternalInput")  # [sub*128+p, kk]
    esel_p = nc.dram_tensor("esel", [128, E], f32, kind="ExternalInput")       # one-hot row c, replicated
    tokp1_p = nc.dram_tensor("tokp1", [16, 256], mybir.dt.int16, kind="ExternalInput")  # token id + 1

    out_p = nc.dram_tensor("out", [OWN, D], f32, kind="ExternalOutput")
    dbg = {}
    if debug:
        for name, shape, dt_ in [
            ("d_xln1T", [D, OWN], f32), ("d_q", [128, 8 * 512], f32), ("d_k", [128, 8 * 512], f32),
            ("d_v", [128, 32 * 132], f32), ("d_oT", [128, 8 * 512], f32), ("d_xoT", [D, OWN], f32),
            ("d_xln2T", [D, OWN], f32), ("d_probs", [OWN, E], f32), ("d_rsum", [128, 64], f32),
            ("d_attnT", [128, 16 * 512], f32), ("d_selg", [TOK, 2], f32),
            ("d_ids", [32, 128], mybir.dt.int16), ("d_gs", [1, 1536], mybir.dt.float16),
            ("d_ns", [OWN, D], bf16),
        ]:
            dbg[name] = nc.dram_tensor(name, shape, dt_, kind="ExternalOutput")

    KT = D // 128  # 8 contraction tiles over D

    with tile.TileContext(nc) as tc:
        # ---------------- DRAM bounce buffers ------------------------------
        with tc.tile_pool(name="dram", bufs=1, space="DRAM") as dram:
            ag1_in = dram.tile([D, OWN], f32r)                    # xln1T contribution
            ag1_out = dram.tile([NC * D, OWN], f32r, addr_space="Shared")
            a2ao_in = dram.tile([NC * 128, OWN], f32r)            # my heads' oT per owner block
            a2ao_out = dram.tile([NC * 128, OWN], f32r)           # full oT for my tokens
            agx_in = dram.tile([OWN, D], bf16)                    # xln2 rows bf16
            agx_out = dram.tile([TOK, D], bf16, addr_space="Shared")
            agp_in = dram.tile([TOK, 2], f32)                     # own toks x all experts [sel, gate]
            agp_out = dram.tile([TOK, 2], f32)
            idx_dram = dram.tile([32, 128], mybir.dt.int16)       # ids bounce (g in 0:16, s in 16:32)
            gs_dram = dram.tile([1, 1536], mybir.dt.float16)      # gate-per-slot bounce
            partial = dram.tile([TOK + 128, D], bf16)             # scatter table (+trash rows)
            rs2_out = dram.tile([OWN, D], bf16)
            x2f_dram = dram.tile([OWN, D], f32)                   # LN2 rows f32 (for P6)
            rt_dram = dram.tile([16, 128], f32r)                  # recip flatten bounce
            gt_dram = dram.tile([4, 128], f32r)                   # gate flatten bounce

            # ---------------- persistent SBUF ------------------------------
            with tc.tile_pool(name="persist", bufs=1) as pp:
                ident = pp.tile([128, 128], f32r)
                nc.sync.dma_start(ident[:], ident_p[:])
                ident_bf = pp.tile([128, 128], bf16)
                nc.vector.tensor_copy(ident_bf[:], ident[:])
                ones = pp.tile([128, 128], f32r)
                nc.sync.dma_start(ones[:], ones_p[:])
                causal = pp.tile([128, 2, 256], f32)
                nc.sync.dma_start(causal[:], causal_p[:].rearrange("(s p) k -> p s k", p=128))
                ln2w = pp.tile([128, KT], f32)
                nc.sync.dma_start(ln2w[:], ln2_p[:])
                esel = pp.tile([128, E], f32)
                nc.sync.dma_start(esel[:], esel_p[:])
                xln2 = pp.tile([128, KT, OWN], f32r)              # LN2 output (own)

                pa_cm = tc.tile_pool(name="phaseA", bufs=1)
                pa = pa_cm.__enter__()

                # ---------- helper: layernorm in [feat, tok] layout ----------
                def layer_norm_T(src, dst, wcol, psum_pool, sbuf_pool):
                    """src, dst: [128, KT, OWN] (f32-readable); wcol [128, KT]."""
                    sum_ps = psum_pool.tile([1, OWN], f32, space="PSUM")
                    sq_ps = psum_pool.tile([1, OWN], f32, space="PSUM")
                    for kt in range(KT):
                        nc.tensor.matmul(sum_ps[:], ones[:, :1], src[:, kt, :],
                                         start=(kt == 0), stop=(kt == KT - 1))
                    for kt in range(KT):
                        sqt = sbuf_pool.tile([128, OWN], f32r, tag="lnsq", bufs=2)
                        nc.vector.tensor_tensor(out=sqt[:], in0=src[:, kt, :], in1=src[:, kt, :],
                                                op=mybir.AluOpType.mult)
                        nc.tensor.matmul(sq_ps[:], ones[:, :1], sqt[:],
                                         start=(kt == 0), stop=(kt == KT - 1))
                    mu = sbuf_pool.tile([1, OWN], f32, tag="lnmu")
                    nc.vector.tensor_scalar_mul(mu[:], sum_ps[:], 1.0 / D)
                    msq = sbuf_pool.tile([1, OWN], f32, tag="lnmsq")
                    nc.vector.tensor_scalar_mul(msq[:], sq_ps[:], 1.0 / D)
                    mu2 = sbuf_pool.tile([1, OWN], f32, tag="lnmu2")
                    nc.vector.tensor_tensor(out=mu2[:], in0=mu[:], in1=mu[:], op=mybir.AluOpType.mult)
                    var = sbuf_pool.tile([1, OWN], f32, tag="lnvar")
                    nc.vector.tensor_sub(var[:], msq[:], mu2[:])
                    nc.vector.tensor_scalar_add(var[:], var[:], EPS)
                    std = sbuf_pool.tile([1, OWN], f32, tag="lnstd")
                    nc.scalar.activation(std[:], var[:], mybir.ActivationFunctionType.Sqrt)
                    rstd = sbuf_pool.tile([1, OWN], f32, tag="lnrstd")
                    nc.vector.reciprocal(rstd[:], std[:])
                    mur = sbuf_pool.tile([1, OWN], f32r, tag="lnmur")
                    nc.vector.tensor_copy(mur[:], mu[:])
                    rstdr = sbuf_pool.tile([1, OWN], f32r, tag="lnrstdr")
                    nc.vector.tensor_copy(rstdr[:], rstd[:])
                    mu_b = psum_pool.tile([128, OWN], f32, space="PSUM")
                    rstd_b = psum_pool.tile([128, OWN], f32, space="PSUM")
                    nc.tensor.matmul(mu_b[:], ones[:1, :], mur[:], start=True, stop=True)
                    nc.tensor.matmul(rstd_b[:], ones[:1, :], rstdr[:], start=True, stop=True)
                    for kt in range(KT):
                        t1 = sbuf_pool.tile([128, OWN], f32, tag="lnt1")
                        nc.vector.tensor_sub(t1[:], src[:, kt, :], mu_b[:])
                        t2 = sbuf_pool.tile([128, OWN], f32, tag="lnt2")
                        nc.vector.tensor_tensor(out=t2[:], in0=t1[:], in1=rstd_b[:], op=mybir.AluOpType.mult)
                        nc.vector.tensor_scalar_mul(dst[:, kt, :], t2[:], wcol[:, kt:kt + 1])

                # ================= P0: LN1 + AG ===========================
                with tc.tile_pool(name="p0sb", bufs=1) as p0sb, \
                     tc.tile_pool(name="p0ps", bufs=1, space="PSUM") as p0ps:
                    ln1w = p0sb.tile([128, KT], f32)
                    nc.sync.dma_start(ln1w[:], ln1_p[:])
                    xt = p0sb.tile([128, KT, OWN], f32r)          # own xT
                    nc.sync.dma_start(xt[:], xT_p[:].rearrange("(kt p) t -> p kt t", p=128))
                    xln1 = p0sb.tile([128, KT, OWN], f32r)
                    layer_norm_T(xt, xln1, ln1w, p0ps, p0sb)
                    nc.sync.dma_start(ag1_in[:].rearrange("(kt p) t -> p kt t", p=128), xln1[:])
                    if debug:
                        nc.sync.dma_start(dbg["d_xln1T"][:].rearrange("(kt p) t -> p kt t", p=128), xln1[:].bitcast(f32))
                nc.gpsimd.collective_compute(
                    "AllGather", mybir.AluOpType.bypass, replica_groups=RG,
                    ins=[ag1_in[:].opt()], outs=[ag1_out[:].opt()])
                # zero the MoE scatter table (runs early, overlaps attention)
                with tc.tile_pool(name="zpool", bufs=1) as zp:
                    zt = zp.tile([128, D], bf16)
                    nc.vector.memset(zt[:], 0)
                    for zi in range((TOK + 128) // 128):
                        nc.sync.dma_start(partial[zi * 128:(zi + 1) * 128, :], zt[:])

                # ================= P1: qkv (own 2 heads, all tokens) =======

                with tc.tile_pool(name="attn_sb", bufs=1) as asb:
                    p1ps_cm = tc.tile_pool(name="p1ps", bufs=1, space="PSUM")
                    aps = p1ps_cm.__enter__()
                    tps = aps
                    wqp_cm = tc.tile_pool(name="wqp", bufs=1)
                    wqp = wqp_cm.__enter__()
                    wq = wqp.tile([128, KT, HPC * HD], f32r)
                    nc.sync.dma_start(wq[:], wq_p[:].rearrange("(kt p) m -> p kt m", p=128))
                    wk = wqp.tile([128, KT, HPC * HD], f32r)
                    nc.sync.dma_start(wk[:], wk_p[:].rearrange("(kt p) m -> p kt m", p=128))
                    wv = wqp.tile([128, KT, HPC * HD], f32r)
                    nc.sync.dma_start(wv[:], wv_p[:].rearrange("(kt p) m -> p kt m", p=128))
                    q_sb = asb.tile([128, NC, 512], f32r)   # [2h*64, rblk, tok]
                    k_sb = asb.tile([128, NC, 512], f32r)
                    v_sb = asb.tile([128, 32, 132], f32r)   # [tok128, t-tile, h*65+{64 feat, 1 ones}]
                    for _t in range(32):
                        nc.vector.tensor_copy(v_sb[:, _t, 64:65], ones[:, :1])
                        nc.vector.tensor_copy(v_sb[:, _t, 129:130], ones[:, :1])
                    for r in range(NC):
                        xg1_r = wqp.tile([128, KT, 512], f32r, tag="xg1", bufs=2)
                        nc.sync.dma_start(
                            xg1_r[:], ag1_out[r * D:(r + 1) * D, :].rearrange("(kt p) t -> p kt t", p=128))
                        q_ps = aps.tile([128, 512], f32, space="PSUM", tag="qkv", bufs=3)
                        for kt in range(KT):
                            nc.tensor.matmul(q_ps[:HPC * HD, :], wq[:, kt, :], xg1_r[:, kt, :],
                                             start=(kt == 0), stop=(kt == KT - 1))
                        nc.vector.tensor_copy(q_sb[:HPC * HD, r, :], q_ps[:HPC * HD, :])
                        k_ps = aps.tile([128, 512], f32, space="PSUM", tag="qkv", bufs=3)
                        for kt in range(KT):
                            nc.tensor.matmul(k_ps[:HPC * HD, :], wk[:, kt, :], xg1_r[:, kt, :],
                                             start=(kt == 0), stop=(kt == KT - 1))
                        nc.vector.tensor_copy(k_sb[:HPC * HD, r, :], k_ps[:HPC * HD, :])
                        v_ps = aps.tile([128, 512], f32, space="PSUM", tag="qkv", bufs=3)
                        for kt in range(KT):
                            nc.tensor.matmul(v_ps[:HPC * HD, :], wv[:, kt, :], xg1_r[:, kt, :],
                                             start=(kt == 0), stop=(kt == KT - 1))
                        vT_sb = asb.tile([128, 512], f32r, tag="vT", bufs=2)
                        nc.vector.tensor_copy(vT_sb[:HPC * HD, :], v_ps[:HPC * HD, :])
                        # transpose v to [tok, feat]; interleave ones col per head
                        for tt in range(4):
                            v_tps = tps.tile([128, 128], f32r, space="PSUM", tag="vtr", bufs=3)
                            nc.tensor.transpose(v_tps[:], vT_sb[:, tt * 128:(tt + 1) * 128], ident[:])
                            nc.vector.tensor_copy(v_sb[:, r * 4 + tt, 0:64], v_tps[:, 0:64])
                            nc.vector.tensor_copy(v_sb[:, r * 4 + tt, 65:129], v_tps[:, 64:128])
                    if debug:
                        nc.sync.dma_start(dbg["d_q"][:].rearrange("p (r t) -> p r t", r=NC), q_sb[:].bitcast(f32))
                        nc.sync.dma_start(dbg["d_k"][:].rearrange("p (r t) -> p r t", r=NC), k_sb[:].bitcast(f32))
                        nc.sync.dma_start(dbg["d_v"][:].rearrange("p (r t) -> p r t", r=32, t=132), v_sb[:].bitcast(f32))

                    wqp_cm.__exit__(None, None, None)
                    p1ps_cm.__exit__(None, None, None)
                    p2ps_cm = tc.tile_pool(name="p2ps", bufs=1, space="PSUM")
                    aps = p2ps_cm.__enter__()
                    tps = aps
                    # ============= P2: scores/softmax/AV per (b, h) =========
                    oT_sb = asb.tile([128, NC, 512], f32r)   # [2h*64, rblk, tok]
                    for b in range(B):
                        for h in range(HPC):
                            hs = h * HD
                            PT = asb.tile([128, 16, 512], f32r, tag="attnT", bufs=1)
                            for qc in range(4):
                                rq = b * 4 + qc
                                nkt = 4 * qc + 4
                                for kt in range(nkt):
                                    u = kt // 2
                                    ru = b * 4 + u // 2
                                    ik = (u % 2) * 256 + (kt % 2) * 128
                                    qs = max(0, u * 256 - qc * 512)
                                    s_ps = aps.tile([128, 512], f32, space="PSUM", tag="score", bufs=3)
                                    nc.tensor.matmul(s_ps[:, qs:512],
                                                     k_sb[hs:hs + HD, ru, ik:ik + 128],
                                                     q_sb[hs:hs + HD, rq, qs:512],
                                                     start=True, stop=True)
                                    dq = u * 256 - qc * 512   # diag block q-col start
                                    if 0 <= dq < 512:
                                        nc.vector.tensor_add(s_ps[:, dq:dq + 256], s_ps[:, dq:dq + 256],
                                                             causal[:, kt % 2, :])
                                    nc.scalar.activation(PT[:, kt, qs:512], s_ps[:, qs:512],
                                                         mybir.ActivationFunctionType.Exp, scale=0.125)
                                o_ps = aps.tile([128, 512], f32, space="PSUM", tag="avps", bufs=3)
                                for kt in range(nkt):
                                    qs = max(0, (kt // 2) * 256 - qc * 512)
                                    nc.tensor.matmul(
                                        o_ps[:HD + 1, qs:512],
                                        v_sb[:, b * 16 + kt, h * 65:h * 65 + 65],
                                        PT[:, kt, qs:512],
                                        start=(kt == 0), stop=(kt == nkt - 1))
                                rs_row = asb.tile([1, 512], f32, tag="rsrow", bufs=2)
                                nc.vector.reciprocal(rs_row[:], o_ps[HD:HD + 1, :])
                                rcp_row = asb.tile([1, 512], f32r, tag="rcprow", bufs=2)
                                nc.vector.tensor_copy(rcp_row[:], rs_row[:])
                                rb_ps = aps.tile([128, 512], f32, space="PSUM", tag="rbcast", bufs=1)
                                nc.tensor.matmul(rb_ps[:], ones[:1, :], rcp_row[:], start=True, stop=True)
                                rb_sb = asb.tile([128, 512], f32, tag="rbsb", bufs=2)
                                nc.vector.tensor_copy(rb_sb[:], rb_ps[:])
                                nc.vector.tensor_tensor(
                                    out=oT_sb[hs:hs + HD, b * 4 + qc, :],
                                    in0=o_ps[:HD, :], in1=rb_sb[:HD, :], op=mybir.AluOpType.mult)
                    if debug:
                        nc.sync.dma_start(dbg["d_oT"][:].rearrange("p (r t) -> p r t", r=NC), oT_sb[:].bitcast(f32))

                    p2ps_cm.__exit__(None, None, None)
                    # ============= P3: ship oT blocks to token owners =======
                    for r in range(NC):
                        nc.sync.dma_start(a2ao_in[r * 128:(r + 1) * 128, :], oT_sb[:, r, :])
                pa_cm.__exit__(None, None, None)
                PERCAP = 96
                CAP = 16 * PERCAP                                  # 1536 slots
                nc.gpsimd.collective_compute(
                    "AllToAll", mybir.AluOpType.bypass, replica_groups=RG,
                    ins=[a2ao_in[:].opt()], outs=[a2ao_out[:].opt()])

                # ================= P4: residual + LN2 + router ==============
                router_w = pp.tile([128, KT, E], f32r)
                nc.sync.dma_start(router_w[:], router_p[:].rearrange("(kt p) e -> p kt e", p=128))
                with tc.tile_pool(name="p4sb", bufs=1) as p4sb, \
                     tc.tile_pool(name="p4ps", bufs=1, space="PSUM") as p4ps:
                    xres = p4sb.tile([128, KT, OWN], f32r)
                    p4o_cm = tc.tile_pool(name="p4o", bufs=1)
                    p4o = p4o_cm.__enter__()
                    oT_full = p4o.tile([128, KT, OWN], f32r)
                    nc.sync.dma_start(oT_full[:], a2ao_out[:].rearrange("(kt p) t -> p kt t", p=128))
                    for dm in range(KT):
                        pj_ps = p4ps.tile([128, OWN], f32, space="PSUM", tag="proj", bufs=2)
                        for kt in range(KT):
                            wpj_t = p4o.tile([128, 128], f32r, tag="wpjt", bufs=4)
                            nc.sync.dma_start(wpj_t[:], wproj_p[kt * 128:(kt + 1) * 128,
                                                                dm * 128:(dm + 1) * 128])
                            nc.tensor.matmul(pj_ps[:], wpj_t[:], oT_full[:, kt, :],
                                             start=(kt == 0), stop=(kt == KT - 1))
                        xt_t = p4sb.tile([128, OWN], f32r, tag="xtt", bufs=2)
                        nc.sync.dma_start(xt_t[:], xT_p[dm * 128:(dm + 1) * 128, :])
                        nc.vector.tensor_add(xres[:, dm, :], xt_t[:], pj_ps[:])
                    p4o_cm.__exit__(None, None, None)
                    if debug:
                        nc.sync.dma_start(dbg["d_xoT"][:].rearrange("(kt p) t -> p kt t", p=128), xres[:].bitcast(f32))
                    layer_norm_T(xres, xln2, ln2w, p4ps, p4sb)
                    if debug:
                        nc.sync.dma_start(dbg["d_xln2T"][:].rearrange("(kt p) t -> p kt t", p=128), xln2[:].bitcast(f32))
                    # transpose xln2 -> token-row layout (bf16 for gather table, f32 for P6)
                    x2row = p4sb.tile([128, 4, D], bf16)
                    for kt in range(KT):
                        for tt in range(4):
                            x2_tps = p4ps.tile([128, 128], f32r, space="PSUM", tag="x2tr", bufs=1)
                            nc.tensor.transpose(x2_tps[:], xln2[:, kt, tt * 128:(tt + 1) * 128], ident[:])
                            nc.vector.tensor_copy(x2row[:, tt, kt * 128:(kt + 1) * 128], x2_tps[:])
                            x2f_t = p4sb.tile([128, 128], f32, tag="x2ft", bufs=3)
                            nc.vector.tensor_copy(x2f_t[:], x2_tps[:])
                            nc.sync.dma_start(x2f_dram[tt * 128:(tt + 1) * 128, kt * 128:(kt + 1) * 128],
                                              x2f_t[:])
                    nc.sync.dma_start(agx_in[:].rearrange("(tt p) d2 -> p tt d2", p=128), x2row[:])
                    # router: logits [tok, E] for own tokens
                    probs = p4sb.tile([128, 4, E], f32)
                    for mt in range(4):
                        lg_ps = p4ps.tile([128, E], f32, space="PSUM", tag="router", bufs=1)
                        for kt in range(KT):
                            nc.tensor.matmul(lg_ps[:], xln2[:, kt, mt * 128:(mt + 1) * 128],
                                             router_w[:, kt, :], start=(kt == 0), stop=(kt == KT - 1))
                        pex = p4sb.tile([128, E], f32, tag="pex", bufs=2)
                        nc.scalar.activation(pex[:], lg_ps[:], mybir.ActivationFunctionType.Exp)
                        psum_r = p4sb.tile([128, 1], f32, tag="psr", bufs=2)
                        nc.vector.tensor_reduce(psum_r[:], pex[:], axis=mybir.AxisListType.X,
                                                op=mybir.AluOpType.add)
                        prcp = p4sb.tile([128, 1], f32, tag="prcp", bufs=2)
                        nc.vector.reciprocal(prcp[:], psum_r[:])
                        nc.vector.tensor_scalar_mul(probs[:, mt, :], pex[:], prcp[:])
                    # own-token [sel, gate] for EVERY expert, A2A-dispatched
                    selg = p4sb.tile([128, E, 4, 2], f32)
                    for mt in range(4):
                        m8 = p4sb.tile([128, 8], f32, tag="m8", bufs=2)
                        nc.vector.max(out=m8[:], in_=probs[:, mt, :])
                        den = p4sb.tile([128, 1], f32, tag="den", bufs=2)
                        nc.vector.tensor_add(den[:], m8[:, 0:1], m8[:, 1:2])
                        rden = p4sb.tile([128, 1], f32, tag="rden", bufs=2)
                        nc.vector.reciprocal(rden[:], den[:])
                        for e in range(E):
                            pe = probs[:, mt, e:e + 1]
                            nc.vector.tensor_tensor(out=selg[:, e, mt, 0:1], in0=pe, in1=m8[:, 1:2],
                                                    op=mybir.AluOpType.is_ge)
                            g1 = p4sb.tile([128, 1], f32, tag="g1", bufs=2)
                            nc.vector.tensor_tensor(out=g1[:], in0=pe, in1=rden[:],
                                                    op=mybir.AluOpType.mult)
                            nc.vector.tensor_tensor(out=selg[:, e, mt, 1:2], in0=g1[:],
                                                    in1=selg[:, e, mt, 0:1],
                                                    op=mybir.AluOpType.mult)
                    nc.sync.dma_start(agp_in[:].rearrange("(e mt p) o -> p e mt o", p=128, mt=4), selg[:])
                    if debug:
                        nc.sync.dma_start(dbg["d_probs"][:].rearrange("(mt p) e -> p mt e", p=128), probs[:])
                nc.gpsimd.collective_compute(
                    "AllToAll", mybir.AluOpType.bypass, replica_groups=RG,
                    ins=[agp_in[:].opt()], outs=[agp_out[:].opt()])
                nc.gpsimd.collective_compute(
                    "AllGather", mybir.AluOpType.bypass, replica_groups=RG,
                    ins=[agx_in[:].opt()], outs=[agx_out[:].opt()])
                moe_w_cm = tc.tile_pool(name="moe_w", bufs=1)
                moe_w = moe_w_cm.__enter__()
                w1b = moe_w.tile([128, KT, F], bf16)       # [Dpart, kt, F]
                w2b = moe_w.tile([128, F // 128, D], bf16)  # [Fpart, ft, D]
                with tc.tile_pool(name="wconv", bufs=3) as wcp:
                    for kt in range(KT):
                        for ch in range(2):
                            wt = wcp.tile([128, 2048], f32, tag="wc32")
                            nc.sync.dma_start(wt[:], w1_p[kt * 128:(kt + 1) * 128,
                                                          ch * 2048:(ch + 1) * 2048])
                            nc.vector.tensor_copy(w1b[:, kt, ch * 2048:(ch + 1) * 2048], wt[:])
                    for ft in range(F // 128):
                        for ch in range(1):
                            wt = wcp.tile([128, 2048], f32, tag="wc32")
                            nc.sync.dma_start(wt[:, :1024], w2_p[ft * 128:(ft + 1) * 128, :])
                            nc.vector.tensor_copy(w2b[:, ft, :], wt[:, :1024])


                # ================= P5: routed expert (own expert) ===========
                # ---- index build: compact token list for own expert ----
                with tc.tile_pool(name="idx_sb", bufs=1) as isb:
                    selw = isb.tile([16, 256], f32)
                    nc.sync.dma_start(selw[:], agp_out[:, 0:1].rearrange("(p j) o -> p (j o)", p=16))
                    gatew = isb.tile([16, 256], f32)
                    nc.sync.dma_start(gatew[:], agp_out[:, 1:2].rearrange("(p j) o -> p (j o)", p=16))
                    tokp1 = isb.tile([16, 256], mybir.dt.int16)
                    nc.sync.dma_start(tokp1[:], tokp1_p[:])
                    incl = isb.tile([16, 256], f32)
                    nc.vector.tensor_tensor_scan(incl[:], selw[:], selw[:], 0.0,
                                                 op0=mybir.AluOpType.add, op1=mybir.AluOpType.bypass)
                    pos = isb.tile([16, 256], f32)
                    nc.vector.tensor_sub(pos[:], incl[:], selw[:])
                    # pos_m = pos*sel + sel - 1  (-1 for unselected), clamped
                    nc.vector.tensor_tensor(out=pos[:], in0=pos[:], in1=selw[:], op=mybir.AluOpType.mult)
                    nc.vector.tensor_add(pos[:], pos[:], selw[:])
                    nc.vector.tensor_scalar_add(pos[:], pos[:], -1.0)
                    nc.vector.tensor_scalar_min(pos[:], pos[:], float(PERCAP - 1))
                    pos16 = isb.tile([16, 256], mybir.dt.int16)
                    nc.vector.tensor_copy(pos16[:], pos[:])
                    idbuf = isb.tile([16, PERCAP], mybir.dt.int16)
                    nc.gpsimd.local_scatter(idbuf[:], tokp1[:], pos16[:], channels=16,
                                            num_elems=PERCAP, num_idxs=256)
                    gate16 = isb.tile([16, 256], mybir.dt.float16)
                    nc.vector.tensor_copy(gate16[:], gatew[:])
                    gatebuf = isb.tile([16, PERCAP], mybir.dt.float16)
                    nc.gpsimd.local_scatter(gatebuf[:], gate16[:], pos16[:], channels=16,
                                            num_elems=PERCAP, num_idxs=256)
                    # fixups in f32: gather ids = max(id-1, 0); scatter ids = (id==0) ? TOK+p : id-1
                    idf = isb.tile([16, PERCAP], f32)
                    nc.vector.tensor_copy(idf[:], idbuf[:])
                    ise = isb.tile([16, PERCAP], f32)
                    nc.vector.tensor_scalar(ise[:], idf[:], 0.0, scalar2=None,
                                            op0=mybir.AluOpType.is_equal)
                    nc.vector.tensor_scalar_add(idf[:], idf[:], -1.0)
                    gth = isb.tile([16, PERCAP], f32)
                    nc.vector.tensor_scalar_max(gth[:], idf[:], 0.0)
                    idsg16 = isb.tile([16, PERCAP], mybir.dt.int16)
                    nc.vector.tensor_copy(idsg16[:], gth[:])
                    nc.vector.tensor_scalar_mul(ise[:], ise[:], float(TOK + 1))
                    nc.vector.tensor_add(idf[:], idf[:], ise[:])
                    idss16 = isb.tile([16, PERCAP], mybir.dt.int16)
                    nc.vector.tensor_copy(idss16[:], idf[:])
                    nc.sync.dma_start(idx_dram[0:16, 0:PERCAP], idsg16[:])
                    nc.sync.dma_start(idx_dram[16:32, 0:PERCAP], idss16[:])
                    # gate per slot: [16, PERCAP] -> flat [CAP] -> [128, CAP//128]
                    nc.sync.dma_start(
                        gs_dram[:].rearrange("o (i p) -> (o p) i", p=16), gatebuf[:])

                idsg = moe_w.tile([128, PERCAP], mybir.dt.int16)
                idss = moe_w.tile([128, PERCAP], mybir.dt.int16)
                for rep in range(8):
                    nc.sync.dma_start(idsg[rep * 16:(rep + 1) * 16, :], idx_dram[0:16, 0:PERCAP])
                    nc.sync.dma_start(idss[rep * 16:(rep + 1) * 16, :], idx_dram[16:32, 0:PERCAP])
                gslot16 = moe_w.tile([128, CAP // 128], mybir.dt.float16)
                nc.sync.dma_start(gslot16[:], gs_dram[:].rearrange("o (c p) -> (o p) c", p=128))
                gslot = moe_w.tile([128, CAP // 128], f32)
                nc.vector.tensor_copy(gslot[:], gslot16[:])

                with tc.tile_pool(name="moe_sb", bufs=1) as msb, \
                     tc.tile_pool(name="moe_ps", bufs=1, space="PSUM") as mps:
                    NCH = CAP // 512                       # 3 slot chunks of 512
                    for cc in range(NCH):
                        xgT = msb.tile([128, KT, 512], bf16, tag="xgt", bufs=2)
                        nc.gpsimd.dma_gather(
                            out_ap=xgT[:], in_ap=agx_out[:],
                            idxs_ap=idsg[:, cc * 32:(cc + 1) * 32],
                            num_idxs=512, num_idxs_reg=512, elem_size=D, transpose=True)
                        h_sb = msb.tile([128, F // 128, 512], bf16, tag="hsb")
                        for fm in range(F // 128):
                            h_ps = mps.tile([128, 512], f32, space="PSUM", tag="hps", bufs=3)
                            for kt in range(KT):
                                nc.tensor.matmul(h_ps[:], w1b[:, kt, fm * 128:(fm + 1) * 128],
                                                 xgT[:, kt, :], start=(kt == 0), stop=(kt == KT - 1))
                            nc.scalar.activation(h_sb[:, fm, :], h_ps[:],
                                                 mybir.ActivationFunctionType.Gelu)
                        eo_sb = msb.tile([128, 4, D], bf16, tag="eosb", bufs=1)
                        for sl in range(4):
                            for nch in range(2):
                                eo_ps = mps.tile([128, 512], f32, space="PSUM", tag="eops", bufs=3)
                                for ft in range(F // 128):
                                    nc.tensor.matmul(eo_ps[:], h_sb[:, ft, sl * 128:(sl + 1) * 128],
                                                     w2b[:, ft, nch * 512:(nch + 1) * 512],
                                                     start=(ft == 0), stop=(ft == F // 128 - 1))
                                nc.vector.tensor_scalar_mul(
                                    eo_sb[:, sl, nch * 512:(nch + 1) * 512], eo_ps[:],
                                    gslot[:, cc * 4 + sl:cc * 4 + sl + 1])
                        nc.gpsimd.dma_scatter_add(
                            out_ap=partial[:], in_ap=eo_sb[:],
                            idxs_ap=idss[:, cc * 32:(cc + 1) * 32],
                            num_idxs=512, num_idxs_reg=512, elem_size=D)
                moe_w_cm.__exit__(None, None, None)
                nc.gpsimd.collective_compute(
                    "ReduceScatter", mybir.AluOpType.add, replica_groups=RG,
                    ins=[partial[0:TOK, :].opt()], outs=[rs2_out[:].opt()])

                if debug:
                    nc.sync.dma_start(dbg["d_selg"][:], agp_out[:])
                    nc.sync.dma_start(dbg["d_ids"][:], idx_dram[:])
                    nc.sync.dma_start(dbg["d_gs"][:], gs_dram[:])
                    nc.sync.dma_start(dbg["d_ns"][:], rs2_out[:])
                # ================= P6: final residual + output ==============
                with tc.tile_pool(name="p6sb", bufs=2) as p6sb:
                    for tt in range(4):
                        ns_t = p6sb.tile([128, D], bf16, tag="nst")
                        nc.sync.dma_start(ns_t[:], rs2_out[tt * 128:(tt + 1) * 128, :])
                        x2_t = p6sb.tile([128, D], f32, tag="x2t")
                        nc.sync.dma_start(x2_t[:], x2f_dram[tt * 128:(tt + 1) * 128, :])
                        o_t = p6sb.tile([128, D], f32, tag="ot")
                        nc.vector.tensor_add(o_t[:], x2_t[:], ns_t[:])
                        nc.sync.dma_start(out_p[tt * 128:(tt + 1) * 128, :], o_t[:])

    nc.compile()
    return nc


def make_in_maps(inputs):
    x = np.asarray(inputs["x"], dtype=np.float32)
    ln1_w = np.asarray(inputs["ln1_w"], dtype=np.float32)
    wqkv = np.asarray(inputs["wqkv"], dtype=np.float32)
    wproj = np.asarray(inputs["wproj"], dtype=np.float32)
    ln2_w = np.asarray(inputs["ln2_w"], dtype=np.float32)
    router_w = np.asarray(inputs["router_w"], dtype=np.float32)
    w1 = np.asarray(inputs["w1"], dtype=np.float32)
    w2 = np.asarray(inputs["w2"], dtype=np.float32)

    x_flat = x.reshape(TOK, D)
    wq_full, wk_full, wv_full = wqkv[:, :D], wqkv[:, D:2 * D], wqkv[:, 2 * D:]

    ident = np.eye(128, dtype=np.float32)
    ones = np.ones((128, 128), dtype=np.float32)
    # causal mask for diagonal 256-unit: [sub*128+p, kk]: 0 if kk <= sub*128+p else -1e9
    causal = np.full((256, 256), -1e9, dtype=np.float32)  # [s*128+p, qq]: 0 if qq >= s*128+p
    for p in range(256):
        causal[p, p:] = 0.0
    ln1_t = ln1_w.reshape(D // 128, 128).T.copy()   # [p, i]
    ln2_t = ln2_w.reshape(D // 128, 128).T.copy()

    in_maps = []
    for c in range(NC):
        rows = slice(c * OWN, (c + 1) * OWN)
        hcols = slice(c * HPC * HD, (c + 1) * HPC * HD)
        esel = np.zeros((128, E), dtype=np.float32)
        esel[:, c] = 1.0
        tokp1 = (np.arange(16)[:, None] * 256 + np.arange(256)[None, :] + 1).astype(np.int16)
        in_maps.append({
            "xT": np.ascontiguousarray(x_flat[rows].T),
            "wq": np.ascontiguousarray(wq_full[:, hcols]),
            "wk": np.ascontiguousarray(wk_full[:, hcols]),
            "wv": np.ascontiguousarray(wv_full[:, hcols]),
            "wproj": wproj,
            "router_w": router_w,
            "ln1_w": ln1_t,
            "ln2_w": ln2_t,
            "w1": w1[c],
            "w2": w2[c],
            "ident": ident,
            "ones": ones,
            "causal": causal,
            "esel": esel,
            "tokp1": tokp1,
        })
    return in_maps


_NC_CACHE = {}


def run(inputs, debug=False, trace=False):
    key = bool(debug)
    if key not in _NC_CACHE:
        _NC_CACHE[key] = build_nc(debug=debug)
    nc = _NC_CACHE[key]
    in_maps = make_in_maps(inputs)
    res = bass_utils.run_bass_kernel_spmd(nc, in_maps, core_ids=list(range(NC)), trace=trace)
    out = np.empty((TOK, D), dtype=np.float32)
    for c in range(NC):
        out[c * OWN:(c + 1) * OWN] = res.results[c]["out"]
    return out.reshape(B, T, D), res


def kernel(**inputs) -> np.ndarray:
    out, _ = run(inputs, debug=False, trace=False)
    return out
